# revision 15
# baseline (speedup 1.0000x reference)
"""Causal attention (B=4, S=4096, D=64, fp32) on 8 Trainium2 NeuronCores.

Strategy (v4)
-------------
Sharding: 2 cores per batch element; the two cores of a batch split the KV
blocks by parity (even / odd 128-row blocks). Each core computes, for every
query position of its batch, the *unnormalized* attention numerator and the
softmax denominator contribution of its own KV half; the host sums and
divides (exact: softmax with no max-subtraction).

The baseline was ACT-engine bound (~42us of exp) with the PE at mid clock.
v4 splits exp across ACT+DVE and shrinks the PE stream with fp8 DoubleRow
matmuls on both the scores and PV sides:

  - scores, BODY pairs: fp8 DoubleRow matmuls, 4-way row-tiled (32-partition
    row groups, 2 fp8 values packed per partition = contraction 64): FOUR kv
    blocks (2 pairs) computed concurrently in one PE slot. Boundary / tile-1
    pairs keep fp16 scores (row-tiled pairs) for accuracy.
  - exp, BODY pairs -> P in fp8e4m3 scaled by 2^-PSHIFT, either by
      * ACT: activation(Exp, scale=1/8, bias=-PSHIFT*ln2) -> fp8 out, or
      * DVE: Schraudolph bit-trick in ONE tensor_scalar pass:
        uint8(x*log2e + (7-PSHIFT+delta)*8) whose e4m3 bit pattern IS
        ~exp(x/8)*2^-PSHIFT (uint8 saturation maps very negative scores
        to P=0). Split ratio balances ACT vs DVE busy time.
  - exp, BOUNDARY (diagonal) pairs: accurate ACT exp -> fp16; 0/1 causal
    masks multiplied on the otherwise-idle GPSIMD engine. Early rows (few
    kv terms, no error averaging) only ever see this path. Tile 1's single
    body pair also stays fp16 (DVE fp16-domain trick + fp16 V): it feeds
    the lowest-N body rows, where fp8 V quantization error would dominate.
  - PV: body pairs use ONE fp8 DoubleRow matmul per pair (2 kv blocks
    contracted at once): lhsT = [V_even|V_odd|ones] as [128, 2, 65] fp8
    (stride 80 for the step%16 rule), rhs = P pair [128, 2, 512] fp8.
    Row 64 of the accumulator is sum(P) = softmax denominator. Padded keys:
    V rows and ones entries zeroed host-side (exact).
  - The schedule is a flat list of pair jobs, software-pipelined on the PE:
    scores of the next group are emitted before the PV of the current pair,
    so the in-order PE queue never head-of-line blocks on exp.
  - No PE warmup: with DMA+all engines active this device pins the PE at
    mid clock regardless (measured), so warmup matmuls only delay the
    first real pair.
Host: packs per-core fp16/fp8 inputs, combines/normalizes outputs.
"""

import numpy as np
from contextlib import ExitStack

import concourse.tile as tile
from concourse import bacc, mybir
from concourse.bass_utils import run_bass_kernel_spmd

B, S, D = 4, 4096, 64
NCORES = 8
BLK = 128            # kv block rows
QTW = 512            # q tile width
NQT = S // QTW       # 8 q tiles
PAR = S // BLK // 2  # 16 kv blocks per parity half
S2 = PAR * BLK       # 2048 kv columns per core

LN2 = 0.6931471805599453
PSHIFT = 3          # P scaled by 2^-PSHIFT: fp8 saturation needs s>61.3
                    # (max score 59.4) with small fp8-subnormal mass
LOG2E = 1.4426950408889634
TRICK_C1 = 12242.93  # fp16 trick: (15 - PSHIFT + delta)*1024, delta=-0.0440
TRICK_C0 = 184.6649652  # 0.125 * log2(e) * 1024
TRICK8_C1 = 31.651   # fp8 trick: (7 - PSHIFT + delta)*8, delta=-0.0436
DVE_NUM, DVE_DEN = 5, 9  # share of fp8 body pairs on DVE (Bresenham)

_prog_cache = {}


def _build_program():
    if "nc" in _prog_cache:
        return _prog_cache["nc"]
    nc = bacc.Bacc("TRN2", target_bir_lowering=False, debug=False, num_devices=NCORES)
    f32, f16 = mybir.dt.float32, mybir.dt.float16
    u8, f8 = mybir.dt.uint8, mybir.dt.float8e4
    Exp = mybir.ActivationFunctionType.Exp
    MUL, ADD = mybir.AluOpType.mult, mybir.AluOpType.add
    DR = mybir.MatmulPerfMode.DoubleRow

    qt_d = nc.dram_tensor("qt", [2 * D, S], f16, kind="ExternalInput").ap()
    kt_d = nc.dram_tensor("kt", [2 * D, S2], f16, kind="ExternalInput").ap()
    # fp8 DoubleRow score operands: 4 identical 32-partition row groups,
    # 2 packed head-dims per partition ([g*32+r, i, pos] = X[pos, 2r+i])
    q8_d = nc.dram_tensor("q8", [128, 2, S], u8, kind="ExternalInput").ap()
    k8_d = nc.dram_tensor("k8", [128, 2, S2], u8, kind="ExternalInput").ap()
    vp_d = nc.dram_tensor("vp", [BLK, PAR * 65], f16, kind="ExternalInput").ap()
    # body-pair fp8 PV weights: 7 pairs x [V_even | V_odd | ones] packed
    # [128, 14, 80] (pair p at [:, 2p:2p+2, :], cols 0:64 V, col 64 ones)
    v8_d = nc.dram_tensor("v8", [BLK, 14, 80], u8, kind="ExternalInput").ap()
    mk_d = nc.dram_tensor("mk", [BLK, 2 * QTW], f16, kind="ExternalInput").ap()
    out_d = nc.dram_tensor("out", [65, S], f32, kind="ExternalOutput").ap()

    # Process deepest tile first (absorbs pipeline fill), T2 last (its tail
    # pair is a cheap ACT fp8 one). Tile 1 mid-schedule.
    tile_order = [7, 0, 6, 5, 4, 3, 1, 2]

    with tile.TileContext(nc) as tc, ExitStack() as ctx:
        const = ctx.enter_context(tc.tile_pool(name="const", bufs=1))
        p16pool = ctx.enter_context(tc.tile_pool(name="p16", bufs=2))
        p8pool = ctx.enter_context(tc.tile_pool(name="p8", bufs=3))
        opool = ctx.enter_context(tc.tile_pool(name="op", bufs=3))
        sc_ps = ctx.enter_context(tc.tile_pool(name="scps", bufs=3, space="PSUM"))
        out_ps = ctx.enter_context(tc.tile_pool(name="ops", bufs=2, space="PSUM"))

        mk_s = const.tile([BLK, 2 * QTW], f16)
        kt_s = const.tile([2 * D, S2], f16)
        q8_s = const.tile([128, 2, S], u8)
        k8_s = const.tile([128, 2, S2], u8)
        vp_s = const.tile([BLK, PAR * 65], f16)
        v8_s = const.tile([BLK, 14, 80], u8)
        qt_s = const.tile([2 * D, S], f16)

        # Input DMAs: few, large, in first-use order. The first body groups
        # (tile 7, kv blocks 0..) need k8 cols 0:512 + q8 tile 7 first.
        T0 = tile_order[0]
        nc.scalar.dma_start(k8_s[:, :, 0:512], k8_d[:, :, 0:512])
        nc.sync.dma_start(q8_s[:, :, T0 * QTW : (T0 + 1) * QTW],
                          q8_d[:, :, T0 * QTW : (T0 + 1) * QTW])
        nc.gpsimd.dma_start(v8_s[:], v8_d[:])
        nc.scalar.dma_start(k8_s[:, :, 512:], k8_d[:, :, 512:])
        nc.sync.dma_start(q8_s[:, :, 0 : T0 * QTW], q8_d[:, :, 0 : T0 * QTW])
        nc.scalar.dma_start(kt_s[:], kt_d[:])         # boundary fp16 K
        nc.sync.dma_start(qt_s[:, T0 * QTW :], qt_d[:, T0 * QTW :])
        nc.sync.dma_start(qt_s[:, 0 : T0 * QTW], qt_d[:, 0 : T0 * QTW])
        nc.gpsimd.dma_start(vp_s[:], vp_d[:])
        nc.scalar.dma_start(mk_s[:], mk_d[:])

        bias_t = const.tile([BLK, 1], f32, name="biasln2")
        nc.gpsimd.memset(bias_t[:], -PSHIFT * LN2)

        # ---- schedule construction -------------------------------------
        jobs = []
        dve_err = 0
        for ti, T in enumerate(tile_order):
            depth = 2 * T + 2
            body = list(range(0, depth - 2, 2))
            pair_lo = body + [depth - 2] if ti == 0 else [depth - 2] + body
            for pi, lo in enumerate(pair_lo):
                if lo == depth - 2:
                    kind = "bnd"
                elif T == 1:
                    kind = "fp16"
                elif ti == len(tile_order) - 1 and pi == len(pair_lo) - 1:
                    kind = "act8"  # cheap tail
                else:
                    dve_err += DVE_NUM
                    if dve_err >= DVE_DEN:
                        dve_err -= DVE_DEN
                        kind = "dve8"
                    else:
                        kind = "act8"
                jobs.append(dict(T=T, ti=ti, lo=lo, kind=kind,
                                 last=pi == len(pair_lo) - 1))
        # score groups: consecutive fp8-score body jobs of one tile pair up
        # (2 pairs = 4 kv blocks in one 4-way DoubleRow PE slot)
        groups = []
        i = 0
        while i < len(jobs):
            j = jobs[i]
            if j["kind"] in ("bnd", "fp16"):
                groups.append([i]); i += 1
            elif (i + 1 < len(jobs) and jobs[i + 1]["kind"] in ("act8", "dve8")
                  and jobs[i + 1]["T"] == j["T"]):
                groups.append([i, i + 1]); i += 2
            else:
                groups.append([i]); i += 1

        n_total = {}
        for j in jobs:
            n_total[j["T"]] = n_total.get(j["T"], 0) + (2 if j["kind"] in ("bnd", "fp16") else 1)

        ops_t, n_mm2 = {}, {}
        copy_ctr = 0

        def emit_scores_group(g):
            """fp8 DoubleRow scores for 1-2 body pairs (2 or 4 kv blocks),
            one 32-partition row group per block."""
            for gi, ji in enumerate(g):
                j = jobs[ji]
                j["sc"] = sc_ps.tile([BLK, 2 * QTW], f32, tag="sc", name=f"scg{ji}")
                j["wid"] = (QTW, QTW)
            blocks = [(jobs[ji], k) for ji in g for k in (0, 1)]
            for bi, (j, k) in enumerate(blocks):
                blk = j["lo"] + k
                nc.tensor.matmul(
                    j["sc"][:, k * QTW : (k + 1) * QTW],
                    k8_s[bi * 32 : (bi + 1) * 32, :, blk * BLK : (blk + 1) * BLK].bitcast(f8),
                    q8_s[bi * 32 : (bi + 1) * 32, :, j["T"] * QTW : (j["T"] + 1) * QTW].bitcast(f8),
                    start=True, stop=True,
                    tile_position=(bi * 32, 0),
                    perf_mode=DR,
                )

        def emit_scores_f16(j):
            """fp16 row-tiled score pair (boundary / tile-1 jobs)."""
            T, lo = j["T"], j["lo"]
            boundary = j["kind"] == "bnd"
            sc = sc_ps.tile([BLK, 2 * QTW], f32, tag="sc")
            wid = (QTW, QTW // 2) if boundary else (QTW, QTW)
            for k, rg in ((0, 0), (1, D)):
                blk = lo + k
                nc.tensor.matmul(
                    sc[:, k * QTW : k * QTW + wid[k]],
                    kt_s[rg : rg + D, blk * BLK : (blk + 1) * BLK],
                    qt_s[rg : rg + D, T * QTW + (QTW - wid[k]) : (T + 1) * QTW],
                    start=True, stop=True,
                    tile_position=(rg, 0),
                )
            j["sc"], j["wid"] = sc, wid

        def emit_scores(g):
            if jobs[g[0]]["kind"] in ("bnd", "fp16"):
                emit_scores_f16(jobs[g[0]])
            else:
                emit_scores_group(g)

        def emit_rest(j):
            nonlocal copy_ctr
            T, lo, sc, wid = j["T"], j["lo"], j["sc"], j["wid"]
            if T not in ops_t:
                ops_t[T] = out_ps.tile([65, QTW], f32, tag="ops", name=f"ops{T}")
                n_mm2[T] = 0
            ops = ops_t[T]

            def pv(out_sl, lhsT, rhs, dr=False):
                n_mm2[T] += 1
                nc.tensor.matmul(
                    out_sl, lhsT, rhs,
                    start=(n_mm2[T] == 1), stop=(n_mm2[T] == n_total[T]),
                    perf_mode=DR if dr else None,
                )

            if j["kind"] == "bnd":
                pt = p16pool.tile([BLK, 2 * QTW], f16, tag="pt")
                ew = QTW + wid[1]
                nc.scalar.activation(pt[:, 0:ew], sc[:, 0:ew], Exp, scale=0.125, bias=bias_t[:])
                nc.gpsimd.tensor_mul(pt[:, 0:256], pt[:, 0:256], mk_s[:, 0:256])
                nc.gpsimd.tensor_mul(
                    pt[:, QTW:ew], pt[:, QTW:ew], mk_s[:, QTW + 256 : 2 * QTW]
                )
                for k in ((1, 0) if j["ti"] == 0 else (0, 1)):
                    blk = lo + k
                    pv(ops[:, QTW - wid[k] : QTW],
                       vp_s[:, blk * 65 : (blk + 1) * 65],
                       pt[:, k * QTW : k * QTW + wid[k]])
            elif j["kind"] == "fp16":
                ptd = p16pool.tile([BLK, 2 * QTW], mybir.dt.uint16, tag="ptd")
                nc.vector.tensor_scalar(
                    ptd[:, :], sc[:, :], TRICK_C0, TRICK_C1, MUL, ADD,
                )
                for k in (0, 1):
                    blk = lo + k
                    pv(ops[:, 0:QTW],
                       vp_s[:, blk * 65 : (blk + 1) * 65],
                       ptd.bitcast(f16)[:, k * QTW : (k + 1) * QTW])
            elif j["kind"] == "dve8":
                p8 = p8pool.tile([BLK, 2, QTW], u8, tag="p8")
                nc.vector.tensor_scalar(
                    p8[:, :, :], sc[:, :].rearrange("p (two f) -> p two f", two=2),
                    LOG2E, TRICK8_C1, MUL, ADD,
                )
                pv(ops[:, 0:QTW], v8_s[:, lo : lo + 2, 0:65].bitcast(f8),
                   p8[:, :, :].bitcast(f8), dr=True)
            else:  # act8
                p8 = p8pool.tile([BLK, 2, QTW], u8, tag="p8")
                nc.scalar.activation(
                    p8[:, :, :].bitcast(f8),
                    sc[:, :].rearrange("p (two f) -> p two f", two=2),
                    Exp, scale=0.125, bias=bias_t[:],
                )
                pv(ops[:, 0:QTW], v8_s[:, lo : lo + 2, 0:65].bitcast(f8),
                   p8[:, :, :].bitcast(f8), dr=True)

            if j["last"]:
                osb = opool.tile([65, QTW], f32, tag="osb", name=f"osb{T}")
                final = j["ti"] == len(tile_order) - 1
                if final or copy_ctr % 2 == 1:
                    nc.scalar.copy(osb[:], ops[:])
                else:
                    nc.vector.tensor_copy(osb[:], ops[:])
                copy_ctr += 1
                nc.sync.dma_start(out_d[:, T * QTW : (T + 1) * QTW], osb[:])

        # pipelined emission: scores of group g+1 precede the exp+PV of
        # group g's jobs in the PE stream
        emit_scores(groups[0])
        for gi, g in enumerate(groups):
            if gi + 1 < len(groups):
                emit_scores(groups[gi + 1])
            for ji in g:
                emit_rest(jobs[ji])

    nc.compile()
    _prog_cache["nc"] = nc
    return nc


def _make_masks(h):
    """[128, 1024] fp16 multiplicative (1=keep, 0=masked) masks: two stacked
    tiles for the 2nd-to-last / last parity-kv loop positions of every q tile
    (relative diagonal offsets r = h and r = h + 2)."""
    tri = (np.arange(QTW)[None, :BLK] >= np.arange(BLK)[:, None]).astype(np.float16)
    full = np.zeros((BLK, BLK), dtype=np.float16)
    keep = np.ones((BLK, BLK), dtype=np.float16)

    def mask_for_r(r):
        cols = []
        for cb in range(QTW // BLK):
            if cb < r:
                cols.append(full)
            elif cb == r:
                cols.append(tri)
            else:
                cols.append(keep)
        return np.concatenate(cols, axis=1)  # [128, 512]

    return np.concatenate([mask_for_r(h), mask_for_r(h + 2)], axis=1)


def _pack_dr(x8):
    """[N, 64] fp8 -> [128, 2, N] uint8: 4 identical 32-partition row
    groups, head-dims packed 2 per partition (d = 2r+i)."""
    g = x8.reshape(-1, 32, 2).transpose(1, 2, 0)  # [32, 2, N]
    return np.tile(g, (4, 1, 1))                  # [128, 2, N]


def kernel(query, key, value, padding):
    import ml_dtypes

    f8 = ml_dtypes.float8_e4m3fn
    query = np.asarray(query, dtype=np.float32)
    key = np.asarray(key, dtype=np.float32)
    value = np.asarray(value, dtype=np.float32)
    padding = np.asarray(padding, dtype=bool)

    nc = _build_program()

    in_maps = []
    for c in range(NCORES):
        b, h = divmod(c, 2)
        qt1 = np.ascontiguousarray(query[b].T).astype(np.float16)  # [64, 4096]
        qt = np.concatenate([qt1, qt1], axis=0)  # [128, 4096] row-tiling dup
        blocks = [2 * i + h for i in range(PAR)]
        perm = np.concatenate([np.arange(BLK * j, BLK * (j + 1)) for j in blocks])
        kperm = key[b][perm]  # [2048, 64]
        kt1 = np.ascontiguousarray(kperm.T).astype(np.float16)  # [64, 2048]
        kt = np.concatenate([kt1, kt1], axis=0)  # [128, 2048]
        q8 = _pack_dr(query[b].astype(f8).view(np.uint8))   # [128, 2, 4096]
        k8 = _pack_dr(kperm.astype(f8).view(np.uint8))      # [128, 2, 2048]
        vp = np.zeros((BLK, PAR * 65), dtype=np.float16)
        vblks = []
        for i, j in enumerate(blocks):
            vblk = value[b, BLK * j : BLK * (j + 1), :].copy()
            pblk = padding[b, BLK * j : BLK * (j + 1)]
            vblk[pblk] = 0.0
            ones = np.where(pblk, 0.0, 1.0).astype(np.float32)
            vp[:, 65 * i : 65 * i + 64] = vblk
            vp[:, 65 * i + 64] = ones
            vblks.append((vblk, ones))
        v8 = np.zeros((BLK, 14, 80), dtype=f8)
        for p in range(7):
            for s_ in range(2):
                vblk, ones = vblks[2 * p + s_]
                v8[:, 2 * p + s_, 0:64] = vblk.astype(f8)
                v8[:, 2 * p + s_, 64] = ones.astype(f8)
        in_maps.append({
            "qt": qt, "kt": kt, "q8": q8, "k8": k8, "vp": vp,
            "v8": v8.view(np.uint8), "mk": _make_masks(h),
        })

    global _last_in_maps
    _last_in_maps = in_maps
    res = run_bass_kernel_spmd(nc, in_maps, list(range(NCORES)))

    out = np.empty((B, S, D), dtype=np.float32)
    for b in range(B):
        r0 = res.results[2 * b]["out"].astype(np.float64)
        r1 = res.results[2 * b + 1]["out"].astype(np.float64)
        num = r0[:64] + r1[:64]
        den = r0[64] + r1[64]
        out[b] = (num / den).T.astype(np.float32)
    return out


# revision 16
# speedup vs baseline: 1.0717x; 1.0717x over previous
"""Causal attention (B=4, S=4096, D=64, fp32) on 8 Trainium2 NeuronCores.

Strategy (v4)
-------------
Sharding: 2 cores per batch element; the two cores of a batch split the KV
blocks by parity (even / odd 128-row blocks). Each core computes, for every
query position of its batch, the *unnormalized* attention numerator and the
softmax denominator contribution of its own KV half; the host sums and
divides (exact: softmax with no max-subtraction).

The baseline was ACT-engine bound (~42us of exp) with the PE at mid clock.
v4 splits exp across ACT+DVE and shrinks the PE stream with fp8 DoubleRow
matmuls on both the scores and PV sides:

  - scores, BODY pairs: fp8 DoubleRow matmuls, 4-way row-tiled (32-partition
    row groups, 2 fp8 values packed per partition = contraction 64): FOUR kv
    blocks (2 pairs) computed concurrently in one PE slot. Boundary / tile-1
    pairs keep fp16 scores (row-tiled pairs) for accuracy.
  - exp, BODY pairs -> P in fp8e4m3 scaled by 2^-PSHIFT, either by
      * ACT: activation(Exp, scale=1/8, bias=-PSHIFT*ln2) -> fp8 out, or
      * DVE: Schraudolph bit-trick in ONE tensor_scalar pass:
        uint8(x*log2e + (7-PSHIFT+delta)*8) whose e4m3 bit pattern IS
        ~exp(x/8)*2^-PSHIFT (uint8 saturation maps very negative scores
        to P=0). Split ratio balances ACT vs DVE busy time.
  - exp, BOUNDARY (diagonal) pairs: accurate ACT exp -> fp16; 0/1 causal
    masks multiplied on the otherwise-idle GPSIMD engine. Early rows (few
    kv terms, no error averaging) only ever see this path. Tile 1's single
    body pair also stays fp16 (DVE fp16-domain trick + fp16 V): it feeds
    the lowest-N body rows, where fp8 V quantization error would dominate.
  - PV: body pairs use ONE fp8 DoubleRow matmul per pair (2 kv blocks
    contracted at once): lhsT = [V_even|V_odd|ones] as [128, 2, 65] fp8
    (stride 80 for the step%16 rule), rhs = P pair [128, 2, 512] fp8.
    Row 64 of the accumulator is sum(P) = softmax denominator. Padded keys:
    V rows and ones entries zeroed host-side (exact).
  - The schedule is a flat list of pair jobs, software-pipelined on the PE:
    scores of the next group are emitted before the PV of the current pair,
    so the in-order PE queue never head-of-line blocks on exp.
  - No PE warmup: with DMA+all engines active this device pins the PE at
    mid clock regardless (measured), so warmup matmuls only delay the
    first real pair.
Host: packs per-core fp16/fp8 inputs, combines/normalizes outputs.
"""

import numpy as np
from contextlib import ExitStack

import concourse.tile as tile
from concourse import bacc, mybir
from concourse.bass_utils import run_bass_kernel_spmd

B, S, D = 4, 4096, 64
NCORES = 8
BLK = 128            # kv block rows
QTW = 512            # q tile width
NQT = S // QTW       # 8 q tiles
PAR = S // BLK // 2  # 16 kv blocks per parity half
S2 = PAR * BLK       # 2048 kv columns per core

LN2 = 0.6931471805599453
PSHIFT = 3          # P scaled by 2^-PSHIFT: fp8 saturation needs s>61.3
                    # (max score 59.4) with small fp8-subnormal mass
LOG2E = 1.4426950408889634
TRICK_C1 = 12242.93  # fp16 trick: (15 - PSHIFT + delta)*1024, delta=-0.0440
TRICK_C0 = 184.6649652  # 0.125 * log2(e) * 1024
TRICK8_C1 = 31.651   # fp8 trick: (7 - PSHIFT + delta)*8, delta=-0.0436
DVE_NUM, DVE_DEN = 4, 9  # share of later fp8 body pairs on DVE (Bresenham)
FIRST_DVE = 4        # first body pairs forced to DVE (no bias/table dep)
WARMUP_MMS = 5       # PE clock-ramp warmup matmuls (overlap the DMA window)

_prog_cache = {}


def _build_program():
    if "nc" in _prog_cache:
        return _prog_cache["nc"]
    nc = bacc.Bacc("TRN2", target_bir_lowering=False, debug=False, num_devices=NCORES)
    f32, f16 = mybir.dt.float32, mybir.dt.float16
    u8, f8 = mybir.dt.uint8, mybir.dt.float8e4
    Exp = mybir.ActivationFunctionType.Exp
    MUL, ADD = mybir.AluOpType.mult, mybir.AluOpType.add
    DR = mybir.MatmulPerfMode.DoubleRow

    qt_d = nc.dram_tensor("qt", [2 * D, S], f16, kind="ExternalInput").ap()
    kt_d = nc.dram_tensor("kt", [2 * D, S2], f16, kind="ExternalInput").ap()
    vp_d = nc.dram_tensor("vp", [BLK, PAR * 65], f16, kind="ExternalInput").ap()
    # body-pair fp8 PV weights: 7 pairs x [V_even | V_odd | ones] packed
    # [128, 14, 80] (pair p at [:, 2p:2p+2, :], cols 0:64 V, col 64 ones)
    v8_d = nc.dram_tensor("v8", [BLK, 14, 80], u8, kind="ExternalInput").ap()
    mk_d = nc.dram_tensor("mk", [BLK, 2 * QTW], f16, kind="ExternalInput").ap()
    out_d = nc.dram_tensor("out", [65, S], f32, kind="ExternalOutput").ap()

    # Process deepest tile first (absorbs pipeline fill), T2 last (its tail
    # pair is a cheap ACT fp8 one). Tile 1 mid-schedule.
    tile_order = [7, 0, 6, 5, 4, 3, 1, 2]

    with tile.TileContext(nc) as tc, ExitStack() as ctx:
        const = ctx.enter_context(tc.tile_pool(name="const", bufs=1))
        p16pool = ctx.enter_context(tc.tile_pool(name="p16", bufs=2))
        p8pool = ctx.enter_context(tc.tile_pool(name="p8", bufs=3))
        opool = ctx.enter_context(tc.tile_pool(name="op", bufs=3))
        sc_ps = ctx.enter_context(tc.tile_pool(name="scps", bufs=3, space="PSUM"))
        out_ps = ctx.enter_context(tc.tile_pool(name="ops", bufs=2, space="PSUM"))

        mk_s = const.tile([BLK, 2 * QTW], f16)
        kt_s = const.tile([2 * D, S2], f16)
        vp_s = const.tile([BLK, PAR * 65], f16)
        v8_s = const.tile([BLK, 14, 80], u8)
        qt_s = const.tile([2 * D, S], f16)

        # exp bias const, on the otherwise-free vector engine so nothing
        # upstream delays the first ACT/DVE exp
        bias_t = const.tile([BLK, 1], f32, name="biasln2")
        nc.vector.memset(bias_t[:], -PSHIFT * LN2)
        scr_t = const.tile([BLK, 1], f32, name="scr")

        # Input DMAs in first-use order (tile 7 body pairs need kt blocks
        # 0.. and qt tile 7 first). A dummy 1-element exp right after the
        # first DMA issue pulls the ~1.3us ACT table load off the critical
        # path (it would otherwise precede the first real exp).
        T0 = tile_order[0]
        nc.scalar.dma_start(kt_s[:, 0:256], kt_d[:, 0:256])
        nc.scalar.activation(scr_t[:], bias_t[:], Exp, scale=0.125)
        nc.sync.dma_start(qt_s[:, T0 * QTW : (T0 + 1) * QTW],
                          qt_d[:, T0 * QTW : (T0 + 1) * QTW])
        nc.gpsimd.dma_start(v8_s[:], v8_d[:])
        nc.scalar.dma_start(kt_s[:, 256:], kt_d[:, 256:])
        nc.sync.dma_start(qt_s[:, 0 : T0 * QTW], qt_d[:, 0 : T0 * QTW])
        nc.gpsimd.dma_start(vp_s[:], vp_d[:])
        nc.scalar.dma_start(mk_s[:], mk_d[:])

        # PE warmup: ~5us of continuous PE busy (warmups + first real
        # pairs, gap-free) steps the clock up ~2x (measured); the warmups
        # overlap the input-DMA window.
        wsrc = const.tile([BLK, QTW], f16, name="wsrc")
        nc.vector.memset(wsrc[:], 0.0)
        wps = sc_ps.tile([BLK, 2 * QTW], f32, tag="sc", name="wps")
        for _ in range(WARMUP_MMS):
            nc.tensor.matmul(wps[:, 0:QTW], wsrc[:, 0:BLK], wsrc[:], start=True, stop=True)

        # ---- schedule construction -------------------------------------
        jobs = []
        dve_err = 0
        for ti, T in enumerate(tile_order):
            depth = 2 * T + 2
            body = list(range(0, depth - 2, 2))
            pair_lo = body + [depth - 2] if ti == 0 else [depth - 2] + body
            for pi, lo in enumerate(pair_lo):
                if lo == depth - 2:
                    kind = "bnd"
                elif T == 1:
                    kind = "fp16"
                elif ti == len(tile_order) - 1 and pi == len(pair_lo) - 1:
                    kind = "act8"  # cheap tail
                elif len(jobs) < FIRST_DVE:
                    kind = "dve8"  # DVE needs no bias const / act table
                else:
                    dve_err += DVE_NUM
                    if dve_err >= DVE_DEN:
                        dve_err -= DVE_DEN
                        kind = "dve8"
                    else:
                        kind = "act8"
                jobs.append(dict(T=T, ti=ti, lo=lo, kind=kind,
                                 last=pi == len(pair_lo) - 1))
        n_total = {}
        for j in jobs:
            n_total[j["T"]] = n_total.get(j["T"], 0) + (2 if j["kind"] in ("bnd", "fp16") else 1)

        ops_t, n_mm2 = {}, {}
        copy_ctr = 0

        def emit_scores(j):
            """fp16 row-tiled score pair (2 kv blocks concurrently)."""
            T, lo = j["T"], j["lo"]
            boundary = j["kind"] == "bnd"
            sc = sc_ps.tile([BLK, 2 * QTW], f32, tag="sc")
            wid = (QTW, QTW // 2) if boundary else (QTW, QTW)
            for k, rg in ((0, 0), (1, D)):
                blk = lo + k
                nc.tensor.matmul(
                    sc[:, k * QTW : k * QTW + wid[k]],
                    kt_s[rg : rg + D, blk * BLK : (blk + 1) * BLK],
                    qt_s[rg : rg + D, T * QTW + (QTW - wid[k]) : (T + 1) * QTW],
                    start=True, stop=True,
                    tile_position=(rg, 0),
                )
            j["sc"], j["wid"] = sc, wid

        def emit_rest(j):
            nonlocal copy_ctr
            T, lo, sc, wid = j["T"], j["lo"], j["sc"], j["wid"]
            if T not in ops_t:
                ops_t[T] = out_ps.tile([65, QTW], f32, tag="ops", name=f"ops{T}")
                n_mm2[T] = 0
            ops = ops_t[T]

            def pv(out_sl, lhsT, rhs, dr=False):
                n_mm2[T] += 1
                nc.tensor.matmul(
                    out_sl, lhsT, rhs,
                    start=(n_mm2[T] == 1), stop=(n_mm2[T] == n_total[T]),
                    perf_mode=DR if dr else None,
                )

            if j["kind"] == "bnd":
                pt = p16pool.tile([BLK, 2 * QTW], f16, tag="pt")
                ew = QTW + wid[1]
                nc.scalar.activation(pt[:, 0:ew], sc[:, 0:ew], Exp, scale=0.125, bias=bias_t[:])
                nc.gpsimd.tensor_mul(pt[:, 0:256], pt[:, 0:256], mk_s[:, 0:256])
                nc.gpsimd.tensor_mul(
                    pt[:, QTW:ew], pt[:, QTW:ew], mk_s[:, QTW + 256 : 2 * QTW]
                )
                for k in ((1, 0) if j["ti"] == 0 else (0, 1)):
                    blk = lo + k
                    pv(ops[:, QTW - wid[k] : QTW],
                       vp_s[:, blk * 65 : (blk + 1) * 65],
                       pt[:, k * QTW : k * QTW + wid[k]])
            elif j["kind"] == "fp16":
                ptd = p16pool.tile([BLK, 2 * QTW], mybir.dt.uint16, tag="ptd")
                nc.vector.tensor_scalar(
                    ptd[:, :], sc[:, :], TRICK_C0, TRICK_C1, MUL, ADD,
                )
                for k in (0, 1):
                    blk = lo + k
                    pv(ops[:, 0:QTW],
                       vp_s[:, blk * 65 : (blk + 1) * 65],
                       ptd.bitcast(f16)[:, k * QTW : (k + 1) * QTW])
            elif j["kind"] == "dve8":
                p8 = p8pool.tile([BLK, 2, QTW], u8, tag="p8")
                nc.vector.tensor_scalar(
                    p8[:, :, :], sc[:, :].rearrange("p (two f) -> p two f", two=2),
                    LOG2E, TRICK8_C1, MUL, ADD,
                )
                pv(ops[:, 0:QTW], v8_s[:, lo : lo + 2, 0:65].bitcast(f8),
                   p8[:, :, :].bitcast(f8), dr=True)
            else:  # act8
                p8 = p8pool.tile([BLK, 2, QTW], u8, tag="p8")
                nc.scalar.activation(
                    p8[:, :, :].bitcast(f8),
                    sc[:, :].rearrange("p (two f) -> p two f", two=2),
                    Exp, scale=0.125, bias=bias_t[:],
                )
                pv(ops[:, 0:QTW], v8_s[:, lo : lo + 2, 0:65].bitcast(f8),
                   p8[:, :, :].bitcast(f8), dr=True)

            if j["last"]:
                osb = opool.tile([65, QTW], f32, tag="osb", name=f"osb{T}")
                final = j["ti"] == len(tile_order) - 1
                if final or copy_ctr % 2 == 1:
                    nc.scalar.copy(osb[:], ops[:])
                else:
                    nc.vector.tensor_copy(osb[:], ops[:])
                copy_ctr += 1
                nc.sync.dma_start(out_d[:, T * QTW : (T + 1) * QTW], osb[:])

        # pipelined emission, lookahead 2: scores of pairs j+1, j+2 precede
        # the PV of pair j in the in-order PE stream, so the PE never
        # head-of-line blocks waiting for pair j's exp (sc pool has exactly
        # 3 bufs = the 3 live score tiles).
        emit_scores(jobs[0])
        emit_scores(jobs[1])
        for ji, j in enumerate(jobs):
            if ji + 2 < len(jobs):
                emit_scores(jobs[ji + 2])
            emit_rest(j)

    nc.compile()
    _prog_cache["nc"] = nc
    return nc


def _make_masks(h):
    """[128, 1024] fp16 multiplicative (1=keep, 0=masked) masks: two stacked
    tiles for the 2nd-to-last / last parity-kv loop positions of every q tile
    (relative diagonal offsets r = h and r = h + 2)."""
    tri = (np.arange(QTW)[None, :BLK] >= np.arange(BLK)[:, None]).astype(np.float16)
    full = np.zeros((BLK, BLK), dtype=np.float16)
    keep = np.ones((BLK, BLK), dtype=np.float16)

    def mask_for_r(r):
        cols = []
        for cb in range(QTW // BLK):
            if cb < r:
                cols.append(full)
            elif cb == r:
                cols.append(tri)
            else:
                cols.append(keep)
        return np.concatenate(cols, axis=1)  # [128, 512]

    return np.concatenate([mask_for_r(h), mask_for_r(h + 2)], axis=1)


def kernel(query, key, value, padding):
    import ml_dtypes

    f8 = ml_dtypes.float8_e4m3fn
    query = np.asarray(query, dtype=np.float32)
    key = np.asarray(key, dtype=np.float32)
    value = np.asarray(value, dtype=np.float32)
    padding = np.asarray(padding, dtype=bool)

    nc = _build_program()

    in_maps = []
    for c in range(NCORES):
        b, h = divmod(c, 2)
        qt1 = np.ascontiguousarray(query[b].T).astype(np.float16)  # [64, 4096]
        qt = np.concatenate([qt1, qt1], axis=0)  # [128, 4096] row-tiling dup
        blocks = [2 * i + h for i in range(PAR)]
        perm = np.concatenate([np.arange(BLK * j, BLK * (j + 1)) for j in blocks])
        kperm = key[b][perm]  # [2048, 64]
        kt1 = np.ascontiguousarray(kperm.T).astype(np.float16)  # [64, 2048]
        kt = np.concatenate([kt1, kt1], axis=0)  # [128, 2048]
        vp = np.zeros((BLK, PAR * 65), dtype=np.float16)
        vblks = []
        for i, j in enumerate(blocks):
            vblk = value[b, BLK * j : BLK * (j + 1), :].copy()
            pblk = padding[b, BLK * j : BLK * (j + 1)]
            vblk[pblk] = 0.0
            ones = np.where(pblk, 0.0, 1.0).astype(np.float32)
            vp[:, 65 * i : 65 * i + 64] = vblk
            vp[:, 65 * i + 64] = ones
            vblks.append((vblk, ones))
        v8 = np.zeros((BLK, 14, 80), dtype=f8)
        for p in range(7):
            for s_ in range(2):
                vblk, ones = vblks[2 * p + s_]
                v8[:, 2 * p + s_, 0:64] = vblk.astype(f8)
                v8[:, 2 * p + s_, 64] = ones.astype(f8)
        in_maps.append({
            "qt": qt, "kt": kt, "vp": vp,
            "v8": v8.view(np.uint8), "mk": _make_masks(h),
        })

    global _last_in_maps
    _last_in_maps = in_maps
    res = run_bass_kernel_spmd(nc, in_maps, list(range(NCORES)))

    out = np.empty((B, S, D), dtype=np.float32)
    for b in range(B):
        r0 = res.results[2 * b]["out"].astype(np.float64)
        r1 = res.results[2 * b + 1]["out"].astype(np.float64)
        num = r0[:64] + r1[:64]
        den = r0[64] + r1[64]
        out[b] = (num / den).T.astype(np.float32)
    return out


# revision 18
# speedup vs baseline: 1.1392x; 1.0629x over previous
"""Causal attention (B=4, S=4096, D=64, fp32) on 8 Trainium2 NeuronCores.

Strategy (v4)
-------------
Sharding: 2 cores per batch element; the two cores of a batch split the KV
blocks by parity (even / odd 128-row blocks). Each core computes, for every
query position of its batch, the *unnormalized* attention numerator and the
softmax denominator contribution of its own KV half; the host sums and
divides (exact: softmax with no max-subtraction).

The baseline was ACT-engine bound (~42us of exp) with the PE at mid clock.
v4 splits exp across ACT+DVE and shrinks the PE stream with fp8 DoubleRow
matmuls on both the scores and PV sides:

  - scores, BODY pairs: fp8 DoubleRow matmuls, 4-way row-tiled (32-partition
    row groups, 2 fp8 values packed per partition = contraction 64): FOUR kv
    blocks (2 pairs) computed concurrently in one PE slot. Boundary / tile-1
    pairs keep fp16 scores (row-tiled pairs) for accuracy.
  - exp, BODY pairs -> P in fp8e4m3 scaled by 2^-PSHIFT, either by
      * ACT: activation(Exp, scale=1/8, bias=-PSHIFT*ln2) -> fp8 out, or
      * DVE: Schraudolph bit-trick in ONE tensor_scalar pass:
        uint8(x*log2e + (7-PSHIFT+delta)*8) whose e4m3 bit pattern IS
        ~exp(x/8)*2^-PSHIFT (uint8 saturation maps very negative scores
        to P=0). Split ratio balances ACT vs DVE busy time.
  - exp, BOUNDARY (diagonal) pairs: accurate ACT exp -> fp16; 0/1 causal
    masks multiplied on the otherwise-idle GPSIMD engine. Early rows (few
    kv terms, no error averaging) only ever see this path. Tile 1's single
    body pair also stays fp16 (DVE fp16-domain trick + fp16 V): it feeds
    the lowest-N body rows, where fp8 V quantization error would dominate.
  - PV: body pairs use ONE fp8 DoubleRow matmul per pair (2 kv blocks
    contracted at once): lhsT = [V_even|V_odd|ones] as [128, 2, 65] fp8
    (stride 80 for the step%16 rule), rhs = P pair [128, 2, 512] fp8.
    Row 64 of the accumulator is sum(P) = softmax denominator. Padded keys:
    V rows and ones entries zeroed host-side (exact).
  - The schedule is a flat list of pair jobs, software-pipelined on the PE:
    scores of the next group are emitted before the PV of the current pair,
    so the in-order PE queue never head-of-line blocks on exp.
  - No PE warmup: with DMA+all engines active this device pins the PE at
    mid clock regardless (measured), so warmup matmuls only delay the
    first real pair.
Host: packs per-core fp16/fp8 inputs, combines/normalizes outputs.
"""

import numpy as np
from contextlib import ExitStack

import concourse.tile as tile
from concourse import bacc, mybir
from concourse.bass_utils import run_bass_kernel_spmd

B, S, D = 4, 4096, 64
NCORES = 8
BLK = 128            # kv block rows
QTW = 512            # q tile width
NQT = S // QTW       # 8 q tiles
PAR = S // BLK // 2  # 16 kv blocks per parity half
S2 = PAR * BLK       # 2048 kv columns per core

LN2 = 0.6931471805599453
PSHIFT = 3          # P scaled by 2^-PSHIFT: fp8 saturation needs s>61.3
                    # (max score 59.4) with small fp8-subnormal mass
LOG2E = 1.4426950408889634
TRICK_C1 = 12242.93  # fp16 trick: (15 - PSHIFT + delta)*1024, delta=-0.0440
TRICK_C0 = 184.6649652  # 0.125 * log2(e) * 1024
TRICK8_C1 = 31.651   # fp8 trick: (7 - PSHIFT + delta)*8, delta=-0.0436
DVE_NUM, DVE_DEN = 5, 9  # share of later fp8 body pairs on DVE (Bresenham)
FIRST_DVE = 4        # first body pairs forced to DVE (no bias/table dep)
WARMUP_MMS = 4       # PE clock-ramp warmup matmuls (overlap the DMA window)

_prog_cache = {}


def _build_program():
    if "nc" in _prog_cache:
        return _prog_cache["nc"]
    nc = bacc.Bacc("TRN2", target_bir_lowering=False, debug=False, num_devices=NCORES)
    f32, f16 = mybir.dt.float32, mybir.dt.float16
    u8, f8 = mybir.dt.uint8, mybir.dt.float8e4
    Exp = mybir.ActivationFunctionType.Exp
    MUL, ADD = mybir.AluOpType.mult, mybir.AluOpType.add
    DR = mybir.MatmulPerfMode.DoubleRow

    qt_d = nc.dram_tensor("qt", [2 * D, S], f16, kind="ExternalInput").ap()
    kt_d = nc.dram_tensor("kt", [2 * D, S2], f16, kind="ExternalInput").ap()
    vp_d = nc.dram_tensor("vp", [BLK, PAR * 65], f16, kind="ExternalInput").ap()
    # body-pair fp8 PV weights: 7 pairs x [V_even | V_odd | ones] packed
    # [128, 14, 80] (pair p at [:, 2p:2p+2, :], cols 0:64 V, col 64 ones)
    v8_d = nc.dram_tensor("v8", [BLK, 14, 80], u8, kind="ExternalInput").ap()
    mk_d = nc.dram_tensor("mk", [BLK, 2 * QTW], f16, kind="ExternalInput").ap()
    out_d = nc.dram_tensor("out", [65, S], f32, kind="ExternalOutput").ap()

    # Process deepest tile first (absorbs pipeline fill), T2 last (its tail
    # pair is a cheap ACT fp8 one). Tile 1 mid-schedule.
    tile_order = [7, 0, 6, 5, 4, 3, 1, 2]

    with tile.TileContext(nc) as tc, ExitStack() as ctx:
        const = ctx.enter_context(tc.tile_pool(name="const", bufs=1))
        p16pool = ctx.enter_context(tc.tile_pool(name="p16", bufs=2))
        p8pool = ctx.enter_context(tc.tile_pool(name="p8", bufs=3))
        opool = ctx.enter_context(tc.tile_pool(name="op", bufs=3))
        sc_ps = ctx.enter_context(tc.tile_pool(name="scps", bufs=3, space="PSUM"))
        out_ps = ctx.enter_context(tc.tile_pool(name="ops", bufs=2, space="PSUM"))

        mk_s = const.tile([BLK, 2 * QTW], f16)
        kt_s = const.tile([2 * D, S2], f16)
        vp_s = const.tile([BLK, PAR * 65], f16)
        v8_s = const.tile([BLK, 14, 80], u8)
        qt_s = const.tile([2 * D, S], f16)

        # exp bias const, on the otherwise-free vector engine so nothing
        # upstream delays the first ACT/DVE exp
        bias_t = const.tile([BLK, 1], f32, name="biasln2")
        nc.vector.memset(bias_t[:], -PSHIFT * LN2)
        scr_t = const.tile([BLK, 1], f32, name="scr")

        # Input DMAs in first-use order (tile 7 body pairs need kt blocks
        # 0.. and qt tile 7 first). A dummy 1-element exp right after the
        # first DMA issue pulls the ~1.3us ACT table load off the critical
        # path (it would otherwise precede the first real exp).
        T0 = tile_order[0]
        nc.scalar.dma_start(kt_s[:, 0:256], kt_d[:, 0:256])
        nc.scalar.activation(scr_t[:], bias_t[:], Exp, scale=0.125)
        nc.sync.dma_start(qt_s[:, T0 * QTW : (T0 + 1) * QTW],
                          qt_d[:, T0 * QTW : (T0 + 1) * QTW])
        nc.gpsimd.dma_start(v8_s[:], v8_d[:])
        nc.scalar.dma_start(kt_s[:, 256:], kt_d[:, 256:])
        nc.sync.dma_start(qt_s[:, 0 : T0 * QTW], qt_d[:, 0 : T0 * QTW])
        nc.gpsimd.dma_start(vp_s[:], vp_d[:])
        nc.scalar.dma_start(mk_s[:], mk_d[:])

        # PE warmup: ~5us of continuous PE busy (warmups + first real
        # pairs, gap-free) steps the clock up ~2x (measured); the warmups
        # overlap the input-DMA window.
        wsrc = const.tile([BLK, QTW], f16, name="wsrc")
        nc.vector.memset(wsrc[:], 0.0)
        wps = sc_ps.tile([BLK, 2 * QTW], f32, tag="sc", name="wps")
        for _ in range(WARMUP_MMS):
            nc.tensor.matmul(wps[:, 0:QTW], wsrc[:, 0:BLK], wsrc[:], start=True, stop=True)

        # ---- schedule construction -------------------------------------
        jobs = []
        dve_err = 0
        for ti, T in enumerate(tile_order):
            depth = 2 * T + 2
            body = list(range(0, depth - 2, 2))
            pair_lo = body + [depth - 2] if ti == 0 else [depth - 2] + body
            for pi, lo in enumerate(pair_lo):
                if lo == depth - 2:
                    kind = "bnd"
                elif T == 1:
                    kind = "fp16"
                elif ti == len(tile_order) - 1 and pi == len(pair_lo) - 1:
                    kind = "act8"  # cheap tail
                elif len(jobs) < FIRST_DVE:
                    kind = "dve8"  # DVE needs no bias const / act table
                else:
                    dve_err += DVE_NUM
                    if dve_err >= DVE_DEN:
                        dve_err -= DVE_DEN
                        kind = "dve8"
                    else:
                        kind = "act8"
                jobs.append(dict(T=T, ti=ti, lo=lo, kind=kind,
                                 last=pi == len(pair_lo) - 1))
        n_total = {}
        for j in jobs:
            n_total[j["T"]] = n_total.get(j["T"], 0) + (2 if j["kind"] in ("bnd", "fp16") else 1)

        ops_t, n_mm2 = {}, {}
        copy_ctr = 0

        def emit_scores(j):
            """fp16 row-tiled score pair (2 kv blocks concurrently)."""
            T, lo = j["T"], j["lo"]
            boundary = j["kind"] == "bnd"
            sc = sc_ps.tile([BLK, 2 * QTW], f32, tag="sc")
            wid = (QTW, QTW // 2) if boundary else (QTW, QTW)
            for k, rg in ((0, 0), (1, D)):
                blk = lo + k
                nc.tensor.matmul(
                    sc[:, k * QTW : k * QTW + wid[k]],
                    kt_s[rg : rg + D, blk * BLK : (blk + 1) * BLK],
                    qt_s[rg : rg + D, T * QTW + (QTW - wid[k]) : (T + 1) * QTW],
                    start=True, stop=True,
                    tile_position=(rg, 0),
                )
            j["sc"], j["wid"] = sc, wid

        final_T = tile_order[-1]
        deferred = []  # (target_ji, pv_thunk): boundary PVs emitted late

        def pv(T, ti, out_cols, lhsT, rhs, dr=False):
            nonlocal copy_ctr
            if T not in ops_t:
                ops_t[T] = out_ps.tile([65, QTW], f32, tag="ops", name=f"ops{T}")
                n_mm2[T] = 0
            ops = ops_t[T]
            n_mm2[T] += 1
            done = n_mm2[T] == n_total[T]
            nc.tensor.matmul(
                ops[:, out_cols[0] : out_cols[1]], lhsT, rhs,
                start=(n_mm2[T] == 1), stop=done,
                perf_mode=DR if dr else None,
            )
            if done:
                osb = opool.tile([65, QTW], f32, tag="osb", name=f"osb{T}")
                if T == final_T or copy_ctr % 2 == 1:
                    nc.scalar.copy(osb[:], ops[:])
                else:
                    nc.vector.tensor_copy(osb[:], ops[:])
                copy_ctr += 1
                nc.sync.dma_start(out_d[:, T * QTW : (T + 1) * QTW], osb[:])

        def emit_rest(j, ji):
            T, ti, lo, sc, wid = j["T"], j["ti"], j["lo"], j["sc"], j["wid"]
            if j["kind"] == "bnd":
                pt = p16pool.tile([BLK, 2 * QTW], f16, tag="pt")
                ew = QTW + wid[1]
                nc.scalar.activation(pt[:, 0:ew], sc[:, 0:ew], Exp, scale=0.125, bias=bias_t[:])
                nc.gpsimd.tensor_mul(pt[:, 0:256], pt[:, 0:256], mk_s[:, 0:256])
                nc.gpsimd.tensor_mul(
                    pt[:, QTW:ew], pt[:, QTW:ew], mk_s[:, QTW + 256 : 2 * QTW]
                )

                def bnd_pv(pt=pt, T=T, ti=ti, lo=lo, wid=wid):
                    for k in (0, 1):
                        pv(T, ti, (QTW - wid[k], QTW),
                           vp_s[:, (lo + k) * 65 : (lo + k + 1) * 65],
                           pt[:, k * QTW : k * QTW + wid[k]])
                # defer the boundary PV so the exp -> gpsimd-mask chain
                # (~2.3us) never stalls the in-order PE queue
                deferred.append((ji + 2, bnd_pv))
            elif j["kind"] == "fp16":
                ptd = p16pool.tile([BLK, 2 * QTW], mybir.dt.uint16, tag="ptd")
                nc.vector.tensor_scalar(
                    ptd[:, :], sc[:, :], TRICK_C0, TRICK_C1, MUL, ADD,
                )
                for k in (0, 1):
                    pv(T, ti, (0, QTW),
                       vp_s[:, (lo + k) * 65 : (lo + k + 1) * 65],
                       ptd.bitcast(f16)[:, k * QTW : (k + 1) * QTW])
            elif j["kind"] == "dve8":
                p8 = p8pool.tile([BLK, 2, QTW], u8, tag="p8")
                nc.vector.tensor_scalar(
                    p8[:, :, :], sc[:, :].rearrange("p (two f) -> p two f", two=2),
                    LOG2E, TRICK8_C1, MUL, ADD,
                )
                pv(T, ti, (0, QTW), v8_s[:, lo : lo + 2, 0:65].bitcast(f8),
                   p8[:, :, :].bitcast(f8), dr=True)
            else:  # act8
                p8 = p8pool.tile([BLK, 2, QTW], u8, tag="p8")
                nc.scalar.activation(
                    p8[:, :, :].bitcast(f8),
                    sc[:, :].rearrange("p (two f) -> p two f", two=2),
                    Exp, scale=0.125, bias=bias_t[:],
                )
                pv(T, ti, (0, QTW), v8_s[:, lo : lo + 2, 0:65].bitcast(f8),
                   p8[:, :, :].bitcast(f8), dr=True)

        # pipelined emission, lookahead 2: scores of pairs j+1, j+2 precede
        # the PV of pair j in the in-order PE stream, so the PE never
        # head-of-line blocks waiting for pair j's exp (sc pool has exactly
        # 3 bufs = the 3 live score tiles).
        emit_scores(jobs[0])
        emit_scores(jobs[1])
        for ji, j in enumerate(jobs):
            if ji + 2 < len(jobs):
                emit_scores(jobs[ji + 2])
            # fire due deferred boundary PVs BEFORE this job's exp (its pt
            # buffer reuse must not precede the deferred reader)
            for tgt, thunk in [dd for dd in deferred if dd[0] <= ji]:
                deferred.remove((tgt, thunk))
                thunk()
            emit_rest(j, ji)
        for _, thunk in deferred:
            thunk()

    nc.compile()
    _prog_cache["nc"] = nc
    return nc


def _make_masks(h):
    """[128, 1024] fp16 multiplicative (1=keep, 0=masked) masks: two stacked
    tiles for the 2nd-to-last / last parity-kv loop positions of every q tile
    (relative diagonal offsets r = h and r = h + 2)."""
    tri = (np.arange(QTW)[None, :BLK] >= np.arange(BLK)[:, None]).astype(np.float16)
    full = np.zeros((BLK, BLK), dtype=np.float16)
    keep = np.ones((BLK, BLK), dtype=np.float16)

    def mask_for_r(r):
        cols = []
        for cb in range(QTW // BLK):
            if cb < r:
                cols.append(full)
            elif cb == r:
                cols.append(tri)
            else:
                cols.append(keep)
        return np.concatenate(cols, axis=1)  # [128, 512]

    return np.concatenate([mask_for_r(h), mask_for_r(h + 2)], axis=1)


def kernel(query, key, value, padding):
    import ml_dtypes

    f8 = ml_dtypes.float8_e4m3fn
    query = np.asarray(query, dtype=np.float32)
    key = np.asarray(key, dtype=np.float32)
    value = np.asarray(value, dtype=np.float32)
    padding = np.asarray(padding, dtype=bool)

    nc = _build_program()

    in_maps = []
    for c in range(NCORES):
        b, h = divmod(c, 2)
        qt1 = np.ascontiguousarray(query[b].T).astype(np.float16)  # [64, 4096]
        qt = np.concatenate([qt1, qt1], axis=0)  # [128, 4096] row-tiling dup
        blocks = [2 * i + h for i in range(PAR)]
        perm = np.concatenate([np.arange(BLK * j, BLK * (j + 1)) for j in blocks])
        kperm = key[b][perm]  # [2048, 64]
        kt1 = np.ascontiguousarray(kperm.T).astype(np.float16)  # [64, 2048]
        kt = np.concatenate([kt1, kt1], axis=0)  # [128, 2048]
        vp = np.zeros((BLK, PAR * 65), dtype=np.float16)
        vblks = []
        for i, j in enumerate(blocks):
            vblk = value[b, BLK * j : BLK * (j + 1), :].copy()
            pblk = padding[b, BLK * j : BLK * (j + 1)]
            vblk[pblk] = 0.0
            ones = np.where(pblk, 0.0, 1.0).astype(np.float32)
            vp[:, 65 * i : 65 * i + 64] = vblk
            vp[:, 65 * i + 64] = ones
            vblks.append((vblk, ones))
        v8 = np.zeros((BLK, 14, 80), dtype=f8)
        for p in range(7):
            for s_ in range(2):
                vblk, ones = vblks[2 * p + s_]
                v8[:, 2 * p + s_, 0:64] = vblk.astype(f8)
                v8[:, 2 * p + s_, 64] = ones.astype(f8)
        in_maps.append({
            "qt": qt, "kt": kt, "vp": vp,
            "v8": v8.view(np.uint8), "mk": _make_masks(h),
        })

    global _last_in_maps
    _last_in_maps = in_maps
    res = run_bass_kernel_spmd(nc, in_maps, list(range(NCORES)))

    out = np.empty((B, S, D), dtype=np.float32)
    for b in range(B):
        r0 = res.results[2 * b]["out"].astype(np.float64)
        r1 = res.results[2 * b + 1]["out"].astype(np.float64)
        num = r0[:64] + r1[:64]
        den = r0[64] + r1[64]
        out[b] = (num / den).T.astype(np.float32)
    return out


# revision 19
# speedup vs baseline: 1.1799x; 1.0358x over previous
"""Causal attention (B=4, S=4096, D=64, fp32) on 8 Trainium2 NeuronCores.

Strategy (v4)
-------------
Sharding: 2 cores per batch element; the two cores of a batch split the KV
blocks by parity (even / odd 128-row blocks). Each core computes, for every
query position of its batch, the *unnormalized* attention numerator and the
softmax denominator contribution of its own KV half; the host sums and
divides (exact: softmax with no max-subtraction).

The baseline was ACT-engine bound (~42us of exp) with the PE at mid clock.
v4 splits exp across ACT+DVE and shrinks the PE stream with fp8 DoubleRow
matmuls on both the scores and PV sides:

  - scores, BODY pairs: fp8 DoubleRow matmuls, 4-way row-tiled (32-partition
    row groups, 2 fp8 values packed per partition = contraction 64): FOUR kv
    blocks (2 pairs) computed concurrently in one PE slot. Boundary / tile-1
    pairs keep fp16 scores (row-tiled pairs) for accuracy.
  - exp, BODY pairs -> P in fp8e4m3 scaled by 2^-PSHIFT, either by
      * ACT: activation(Exp, scale=1/8, bias=-PSHIFT*ln2) -> fp8 out, or
      * DVE: Schraudolph bit-trick in ONE tensor_scalar pass:
        uint8(x*log2e + (7-PSHIFT+delta)*8) whose e4m3 bit pattern IS
        ~exp(x/8)*2^-PSHIFT (uint8 saturation maps very negative scores
        to P=0). Split ratio balances ACT vs DVE busy time.
  - exp, BOUNDARY (diagonal) pairs: accurate ACT exp -> fp16; 0/1 causal
    masks multiplied on the otherwise-idle GPSIMD engine. Early rows (few
    kv terms, no error averaging) only ever see this path. Tile 1's single
    body pair also stays fp16 (DVE fp16-domain trick + fp16 V): it feeds
    the lowest-N body rows, where fp8 V quantization error would dominate.
  - PV: body pairs use ONE fp8 DoubleRow matmul per pair (2 kv blocks
    contracted at once): lhsT = [V_even|V_odd|ones] as [128, 2, 65] fp8
    (stride 80 for the step%16 rule), rhs = P pair [128, 2, 512] fp8.
    Row 64 of the accumulator is sum(P) = softmax denominator. Padded keys:
    V rows and ones entries zeroed host-side (exact).
  - The schedule is a flat list of pair jobs, software-pipelined on the PE:
    scores of the next group are emitted before the PV of the current pair,
    so the in-order PE queue never head-of-line blocks on exp.
  - No PE warmup: with DMA+all engines active this device pins the PE at
    mid clock regardless (measured), so warmup matmuls only delay the
    first real pair.
Host: packs per-core fp16/fp8 inputs, combines/normalizes outputs.
"""

import numpy as np
from contextlib import ExitStack

import concourse.tile as tile
from concourse import bacc, mybir
from concourse.bass_utils import run_bass_kernel_spmd

B, S, D = 4, 4096, 64
NCORES = 8
BLK = 128            # kv block rows
QTW = 512            # q tile width
NQT = S // QTW       # 8 q tiles
PAR = S // BLK // 2  # 16 kv blocks per parity half
S2 = PAR * BLK       # 2048 kv columns per core

LN2 = 0.6931471805599453
PSHIFT = 3          # P scaled by 2^-PSHIFT: fp8 saturation needs s>61.3
                    # (max score 59.4) with small fp8-subnormal mass
LOG2E = 1.4426950408889634
TRICK_C1 = 12242.93  # fp16 trick: (15 - PSHIFT + delta)*1024, delta=-0.0440
TRICK_C0 = 184.6649652  # 0.125 * log2(e) * 1024
TRICK8_C1 = 31.651   # fp8 trick: (7 - PSHIFT + delta)*8, delta=-0.0436
DVE_NUM, DVE_DEN = 5, 9  # share of later fp8 body pairs on DVE (Bresenham)
FIRST_DVE = 2        # first body pairs forced to DVE (no bias/table dep)
WARMUP_MMS = 4       # PE clock-ramp warmup matmuls (overlap the DMA window)

_prog_cache = {}


def _build_program():
    if "nc" in _prog_cache:
        return _prog_cache["nc"]
    nc = bacc.Bacc("TRN2", target_bir_lowering=False, debug=False, num_devices=NCORES)
    f32, f16 = mybir.dt.float32, mybir.dt.float16
    u8, f8 = mybir.dt.uint8, mybir.dt.float8e4
    Exp = mybir.ActivationFunctionType.Exp
    MUL, ADD = mybir.AluOpType.mult, mybir.AluOpType.add
    DR = mybir.MatmulPerfMode.DoubleRow

    qt_d = nc.dram_tensor("qt", [2 * D, S], f16, kind="ExternalInput").ap()
    kt_d = nc.dram_tensor("kt", [2 * D, S2], f16, kind="ExternalInput").ap()
    vp_d = nc.dram_tensor("vp", [BLK, PAR * 65], f16, kind="ExternalInput").ap()
    # body-pair fp8 PV weights: 7 pairs x [V_even | V_odd | ones] packed
    # [128, 14, 80] (pair p at [:, 2p:2p+2, :], cols 0:64 V, col 64 ones)
    v8_d = nc.dram_tensor("v8", [BLK, 14, 80], u8, kind="ExternalInput").ap()
    mk_d = nc.dram_tensor("mk", [BLK, 2 * QTW], f16, kind="ExternalInput").ap()
    out_d = nc.dram_tensor("out", [65, S], f32, kind="ExternalOutput").ap()

    # Process deepest tile first (absorbs pipeline fill), T2 last (its tail
    # pair is a cheap ACT fp8 one). Tile 1 mid-schedule.
    tile_order = [7, 0, 6, 5, 4, 3, 1, 2]

    with tile.TileContext(nc) as tc, ExitStack() as ctx:
        const = ctx.enter_context(tc.tile_pool(name="const", bufs=1))
        p16pool = ctx.enter_context(tc.tile_pool(name="p16", bufs=2))
        p8pool = ctx.enter_context(tc.tile_pool(name="p8", bufs=3))
        opool = ctx.enter_context(tc.tile_pool(name="op", bufs=3))
        sc_ps = ctx.enter_context(tc.tile_pool(name="scps", bufs=3, space="PSUM"))
        out_ps = ctx.enter_context(tc.tile_pool(name="ops", bufs=2, space="PSUM"))

        mk_s = const.tile([BLK, 2 * QTW], f16)
        kt_s = const.tile([2 * D, S2], f16)
        vp_s = const.tile([BLK, PAR * 65], f16)
        v8_s = const.tile([BLK, 14, 80], u8)
        qt_s = const.tile([2 * D, S], f16)

        # exp bias const, on the otherwise-free vector engine so nothing
        # upstream delays the first ACT/DVE exp
        bias_t = const.tile([BLK, 1], f32, name="biasln2")
        nc.vector.memset(bias_t[:], -PSHIFT * LN2)
        scr_t = const.tile([BLK, 1], f32, name="scr")

        # Input DMAs in first-use order (tile 7 body pairs need kt blocks
        # 0.. and qt tile 7 first). A dummy 1-element exp right after the
        # first DMA issue pulls the ~1.3us ACT table load off the critical
        # path (it would otherwise precede the first real exp).
        T0 = tile_order[0]
        nc.scalar.dma_start(kt_s[:, 0:256], kt_d[:, 0:256])
        nc.scalar.activation(scr_t[:], bias_t[:], Exp, scale=0.125)
        nc.sync.dma_start(qt_s[:, T0 * QTW : (T0 + 1) * QTW],
                          qt_d[:, T0 * QTW : (T0 + 1) * QTW])
        nc.gpsimd.dma_start(v8_s[:], v8_d[:])
        nc.scalar.dma_start(kt_s[:, 256:], kt_d[:, 256:])
        nc.sync.dma_start(qt_s[:, 0 : T0 * QTW], qt_d[:, 0 : T0 * QTW])
        nc.gpsimd.dma_start(vp_s[:], vp_d[:])
        nc.scalar.dma_start(mk_s[:], mk_d[:])

        # PE warmup: ~5us of continuous PE busy (warmups + first real
        # pairs, gap-free) steps the clock up ~2x (measured); the warmups
        # overlap the input-DMA window.
        wsrc = const.tile([BLK, QTW], f16, name="wsrc")
        nc.vector.memset(wsrc[:], 0.0)
        wps = sc_ps.tile([BLK, 2 * QTW], f32, tag="sc", name="wps")
        for _ in range(WARMUP_MMS):
            nc.tensor.matmul(wps[:, 0:QTW], wsrc[:, 0:BLK], wsrc[:], start=True, stop=True)

        # ---- schedule construction -------------------------------------
        jobs = []
        dve_err = 0
        for ti, T in enumerate(tile_order):
            depth = 2 * T + 2
            body = list(range(0, depth - 2, 2))
            pair_lo = body + [depth - 2] if ti == 0 else [depth - 2] + body
            for pi, lo in enumerate(pair_lo):
                if lo == depth - 2:
                    kind = "bnd"
                elif T == 1:
                    kind = "fp16"
                elif ti == len(tile_order) - 1 and pi == len(pair_lo) - 1:
                    kind = "act8"  # cheap tail
                elif len(jobs) < FIRST_DVE:
                    kind = "dve8"  # DVE needs no bias const / act table
                else:
                    dve_err += DVE_NUM
                    if dve_err >= DVE_DEN:
                        dve_err -= DVE_DEN
                        kind = "dve8"
                    else:
                        kind = "act8"
                jobs.append(dict(T=T, ti=ti, lo=lo, kind=kind,
                                 last=pi == len(pair_lo) - 1))
        n_total = {}
        for j in jobs:
            n_total[j["T"]] = n_total.get(j["T"], 0) + (2 if j["kind"] in ("bnd", "fp16") else 1)

        ops_t, n_mm2 = {}, {}
        copy_ctr = 0

        def emit_scores(j):
            """fp16 row-tiled score pair (2 kv blocks concurrently)."""
            T, lo = j["T"], j["lo"]
            boundary = j["kind"] == "bnd"
            sc = sc_ps.tile([BLK, 2 * QTW], f32, tag="sc")
            wid = (QTW, QTW // 2) if boundary else (QTW, QTW)
            for k, rg in ((0, 0), (1, D)):
                blk = lo + k
                nc.tensor.matmul(
                    sc[:, k * QTW : k * QTW + wid[k]],
                    kt_s[rg : rg + D, blk * BLK : (blk + 1) * BLK],
                    qt_s[rg : rg + D, T * QTW + (QTW - wid[k]) : (T + 1) * QTW],
                    start=True, stop=True,
                    tile_position=(rg, 0),
                )
            j["sc"], j["wid"] = sc, wid

        final_T = tile_order[-1]
        deferred = []  # (target_ji, pv_thunk): boundary PVs emitted late

        def pv(T, ti, out_cols, lhsT, rhs, dr=False):
            nonlocal copy_ctr
            if T not in ops_t:
                ops_t[T] = out_ps.tile([65, QTW], f32, tag="ops", name=f"ops{T}")
                n_mm2[T] = 0
            ops = ops_t[T]
            n_mm2[T] += 1
            done = n_mm2[T] == n_total[T]
            nc.tensor.matmul(
                ops[:, out_cols[0] : out_cols[1]], lhsT, rhs,
                start=(n_mm2[T] == 1), stop=done,
                perf_mode=DR if dr else None,
            )
            if done:
                osb = opool.tile([65, QTW], f32, tag="osb", name=f"osb{T}")
                if T == final_T or copy_ctr % 2 == 1:
                    nc.scalar.copy(osb[:], ops[:])
                else:
                    nc.vector.tensor_copy(osb[:], ops[:])
                copy_ctr += 1
                nc.sync.dma_start(out_d[:, T * QTW : (T + 1) * QTW], osb[:])

        def emit_rest(j, ji):
            T, ti, lo, sc, wid = j["T"], j["ti"], j["lo"], j["sc"], j["wid"]
            if j["kind"] == "bnd":
                pt = p16pool.tile([BLK, 2 * QTW], f16, tag="pt")
                ew = QTW + wid[1]
                nc.scalar.activation(pt[:, 0:ew], sc[:, 0:ew], Exp, scale=0.125, bias=bias_t[:])
                nc.gpsimd.tensor_mul(pt[:, 0:256], pt[:, 0:256], mk_s[:, 0:256])
                nc.gpsimd.tensor_mul(
                    pt[:, QTW:ew], pt[:, QTW:ew], mk_s[:, QTW + 256 : 2 * QTW]
                )

                def bnd_pv(pt=pt, T=T, ti=ti, lo=lo, wid=wid):
                    for k in (0, 1):
                        pv(T, ti, (QTW - wid[k], QTW),
                           vp_s[:, (lo + k) * 65 : (lo + k + 1) * 65],
                           pt[:, k * QTW : k * QTW + wid[k]])
                # defer the boundary PV so the exp -> gpsimd-mask chain
                # (~2.3us) never stalls the in-order PE queue; the final
                # tile's boundary goes last of all (cheapest tail chain:
                # no exp between the last PV and the output copy)
                tgt = len(jobs) + 1 if T == final_T else ji + 2
                deferred.append((tgt, bnd_pv))
            elif j["kind"] == "fp16":
                ptd = p16pool.tile([BLK, 2 * QTW], mybir.dt.uint16, tag="ptd")
                nc.vector.tensor_scalar(
                    ptd[:, :], sc[:, :], TRICK_C0, TRICK_C1, MUL, ADD,
                )
                for k in (0, 1):
                    pv(T, ti, (0, QTW),
                       vp_s[:, (lo + k) * 65 : (lo + k + 1) * 65],
                       ptd.bitcast(f16)[:, k * QTW : (k + 1) * QTW])
            elif j["kind"] == "dve8":
                p8 = p8pool.tile([BLK, 2, QTW], u8, tag="p8")
                nc.vector.tensor_scalar(
                    p8[:, :, :], sc[:, :].rearrange("p (two f) -> p two f", two=2),
                    LOG2E, TRICK8_C1, MUL, ADD,
                )
                pv(T, ti, (0, QTW), v8_s[:, lo : lo + 2, 0:65].bitcast(f8),
                   p8[:, :, :].bitcast(f8), dr=True)
            else:  # act8
                p8 = p8pool.tile([BLK, 2, QTW], u8, tag="p8")
                nc.scalar.activation(
                    p8[:, :, :].bitcast(f8),
                    sc[:, :].rearrange("p (two f) -> p two f", two=2),
                    Exp, scale=0.125, bias=bias_t[:],
                )
                pv(T, ti, (0, QTW), v8_s[:, lo : lo + 2, 0:65].bitcast(f8),
                   p8[:, :, :].bitcast(f8), dr=True)

        # pipelined emission, lookahead 2: scores of pairs j+1, j+2 precede
        # the PV of pair j in the in-order PE stream, so the PE never
        # head-of-line blocks waiting for pair j's exp (sc pool has exactly
        # 3 bufs = the 3 live score tiles).
        emit_scores(jobs[0])
        emit_scores(jobs[1])
        for ji, j in enumerate(jobs):
            if ji + 2 < len(jobs):
                emit_scores(jobs[ji + 2])
            # fire due deferred boundary PVs BEFORE this job's exp (its pt
            # buffer reuse must not precede the deferred reader)
            for tgt, thunk in [dd for dd in deferred if dd[0] <= ji]:
                deferred.remove((tgt, thunk))
                thunk()
            emit_rest(j, ji)
        for _, thunk in deferred:
            thunk()

    nc.compile()
    _prog_cache["nc"] = nc
    return nc


def _make_masks(h):
    """[128, 1024] fp16 multiplicative (1=keep, 0=masked) masks: two stacked
    tiles for the 2nd-to-last / last parity-kv loop positions of every q tile
    (relative diagonal offsets r = h and r = h + 2)."""
    tri = (np.arange(QTW)[None, :BLK] >= np.arange(BLK)[:, None]).astype(np.float16)
    full = np.zeros((BLK, BLK), dtype=np.float16)
    keep = np.ones((BLK, BLK), dtype=np.float16)

    def mask_for_r(r):
        cols = []
        for cb in range(QTW // BLK):
            if cb < r:
                cols.append(full)
            elif cb == r:
                cols.append(tri)
            else:
                cols.append(keep)
        return np.concatenate(cols, axis=1)  # [128, 512]

    return np.concatenate([mask_for_r(h), mask_for_r(h + 2)], axis=1)


def kernel(query, key, value, padding):
    import ml_dtypes

    f8 = ml_dtypes.float8_e4m3fn
    query = np.asarray(query, dtype=np.float32)
    key = np.asarray(key, dtype=np.float32)
    value = np.asarray(value, dtype=np.float32)
    padding = np.asarray(padding, dtype=bool)

    nc = _build_program()

    in_maps = []
    for c in range(NCORES):
        b, h = divmod(c, 2)
        qt1 = np.ascontiguousarray(query[b].T).astype(np.float16)  # [64, 4096]
        qt = np.concatenate([qt1, qt1], axis=0)  # [128, 4096] row-tiling dup
        blocks = [2 * i + h for i in range(PAR)]
        perm = np.concatenate([np.arange(BLK * j, BLK * (j + 1)) for j in blocks])
        kperm = key[b][perm]  # [2048, 64]
        kt1 = np.ascontiguousarray(kperm.T).astype(np.float16)  # [64, 2048]
        kt = np.concatenate([kt1, kt1], axis=0)  # [128, 2048]
        vp = np.zeros((BLK, PAR * 65), dtype=np.float16)
        vblks = []
        for i, j in enumerate(blocks):
            vblk = value[b, BLK * j : BLK * (j + 1), :].copy()
            pblk = padding[b, BLK * j : BLK * (j + 1)]
            vblk[pblk] = 0.0
            ones = np.where(pblk, 0.0, 1.0).astype(np.float32)
            vp[:, 65 * i : 65 * i + 64] = vblk
            vp[:, 65 * i + 64] = ones
            vblks.append((vblk, ones))
        v8 = np.zeros((BLK, 14, 80), dtype=f8)
        for p in range(7):
            for s_ in range(2):
                vblk, ones = vblks[2 * p + s_]
                v8[:, 2 * p + s_, 0:64] = vblk.astype(f8)
                v8[:, 2 * p + s_, 64] = ones.astype(f8)
        in_maps.append({
            "qt": qt, "kt": kt, "vp": vp,
            "v8": v8.view(np.uint8), "mk": _make_masks(h),
        })

    global _last_in_maps
    _last_in_maps = in_maps
    res = run_bass_kernel_spmd(nc, in_maps, list(range(NCORES)))

    out = np.empty((B, S, D), dtype=np.float32)
    for b in range(B):
        r0 = res.results[2 * b]["out"].astype(np.float64)
        r1 = res.results[2 * b + 1]["out"].astype(np.float64)
        num = r0[:64] + r1[:64]
        den = r0[64] + r1[64]
        out[b] = (num / den).T.astype(np.float32)
    return out


# revision 20
# speedup vs baseline: 1.1844x; 1.0038x over previous
"""Causal attention (B=4, S=4096, D=64, fp32) on 8 Trainium2 NeuronCores.

Strategy (v4)
-------------
Sharding: 2 cores per batch element; the two cores of a batch split the KV
blocks by parity (even / odd 128-row blocks). Each core computes, for every
query position of its batch, the *unnormalized* attention numerator and the
softmax denominator contribution of its own KV half; the host sums and
divides (exact: softmax with no max-subtraction).

The baseline was ACT-engine bound (~42us of exp) with the PE at mid clock.
v4 splits exp across ACT+DVE and shrinks the PE stream with fp8 DoubleRow
matmuls on both the scores and PV sides:

  - scores, BODY pairs: fp8 DoubleRow matmuls, 4-way row-tiled (32-partition
    row groups, 2 fp8 values packed per partition = contraction 64): FOUR kv
    blocks (2 pairs) computed concurrently in one PE slot. Boundary / tile-1
    pairs keep fp16 scores (row-tiled pairs) for accuracy.
  - exp, BODY pairs -> P in fp8e4m3 scaled by 2^-PSHIFT, either by
      * ACT: activation(Exp, scale=1/8, bias=-PSHIFT*ln2) -> fp8 out, or
      * DVE: Schraudolph bit-trick in ONE tensor_scalar pass:
        uint8(x*log2e + (7-PSHIFT+delta)*8) whose e4m3 bit pattern IS
        ~exp(x/8)*2^-PSHIFT (uint8 saturation maps very negative scores
        to P=0). Split ratio balances ACT vs DVE busy time.
  - exp, BOUNDARY (diagonal) pairs: accurate ACT exp -> fp16; 0/1 causal
    masks multiplied on the otherwise-idle GPSIMD engine. Early rows (few
    kv terms, no error averaging) only ever see this path. Tile 1's single
    body pair also stays fp16 (DVE fp16-domain trick + fp16 V): it feeds
    the lowest-N body rows, where fp8 V quantization error would dominate.
  - PV: body pairs use ONE fp8 DoubleRow matmul per pair (2 kv blocks
    contracted at once): lhsT = [V_even|V_odd|ones] as [128, 2, 65] fp8
    (stride 80 for the step%16 rule), rhs = P pair [128, 2, 512] fp8.
    Row 64 of the accumulator is sum(P) = softmax denominator. Padded keys:
    V rows and ones entries zeroed host-side (exact).
  - The schedule is a flat list of pair jobs, software-pipelined on the PE:
    scores of the next group are emitted before the PV of the current pair,
    so the in-order PE queue never head-of-line blocks on exp.
  - No PE warmup: with DMA+all engines active this device pins the PE at
    mid clock regardless (measured), so warmup matmuls only delay the
    first real pair.
Host: packs per-core fp16/fp8 inputs, combines/normalizes outputs.
"""

import numpy as np
from contextlib import ExitStack

import concourse.tile as tile
from concourse import bacc, mybir
from concourse.bass_utils import run_bass_kernel_spmd

B, S, D = 4, 4096, 64
NCORES = 8
BLK = 128            # kv block rows
QTW = 512            # q tile width
NQT = S // QTW       # 8 q tiles
PAR = S // BLK // 2  # 16 kv blocks per parity half
S2 = PAR * BLK       # 2048 kv columns per core

LN2 = 0.6931471805599453
PSHIFT = 3          # P scaled by 2^-PSHIFT: fp8 saturation needs s>61.3
                    # (max score 59.4) with small fp8-subnormal mass
LOG2E = 1.4426950408889634
TRICK_C1 = 12242.93  # fp16 trick: (15 - PSHIFT + delta)*1024, delta=-0.0440
TRICK_C0 = 184.6649652  # 0.125 * log2(e) * 1024
TRICK8_C1 = 31.651   # fp8 trick: (7 - PSHIFT + delta)*8, delta=-0.0436
DVE_NUM, DVE_DEN = 5, 9  # share of later fp8 body pairs on DVE (Bresenham)
FIRST_DVE = 4        # first body pairs alternate DVE/ACT (parallel fill)
WARMUP_MMS = 5       # PE clock-ramp warmup matmuls (bridge to data-ready)

_prog_cache = {}


def _build_program():
    if "nc" in _prog_cache:
        return _prog_cache["nc"]
    nc = bacc.Bacc("TRN2", target_bir_lowering=False, debug=False, num_devices=NCORES)
    f32, f16 = mybir.dt.float32, mybir.dt.float16
    u8, f8 = mybir.dt.uint8, mybir.dt.float8e4
    Exp = mybir.ActivationFunctionType.Exp
    MUL, ADD = mybir.AluOpType.mult, mybir.AluOpType.add
    DR = mybir.MatmulPerfMode.DoubleRow

    qt_d = nc.dram_tensor("qt", [2 * D, S], f16, kind="ExternalInput").ap()
    kt_d = nc.dram_tensor("kt", [2 * D, S2], f16, kind="ExternalInput").ap()
    vp_d = nc.dram_tensor("vp", [BLK, PAR * 65], f16, kind="ExternalInput").ap()
    # body-pair fp8 PV weights: 7 pairs x [V_even | V_odd | ones] packed
    # [128, 14, 80] (pair p at [:, 2p:2p+2, :], cols 0:64 V, col 64 ones)
    v8_d = nc.dram_tensor("v8", [BLK, 14, 80], u8, kind="ExternalInput").ap()
    mk_d = nc.dram_tensor("mk", [BLK, 2 * QTW], f16, kind="ExternalInput").ap()
    out_d = nc.dram_tensor("out", [65, S], f32, kind="ExternalOutput").ap()

    # Process deepest tile first (absorbs pipeline fill), T2 last (its tail
    # pair is a cheap ACT fp8 one). Tile 1 mid-schedule.
    tile_order = [7, 0, 6, 5, 4, 3, 1, 2]

    with tile.TileContext(nc) as tc, ExitStack() as ctx:
        const = ctx.enter_context(tc.tile_pool(name="const", bufs=1))
        p16pool = ctx.enter_context(tc.tile_pool(name="p16", bufs=2))
        p8pool = ctx.enter_context(tc.tile_pool(name="p8", bufs=3))
        opool = ctx.enter_context(tc.tile_pool(name="op", bufs=3))
        sc_ps = ctx.enter_context(tc.tile_pool(name="scps", bufs=3, space="PSUM"))
        out_ps = ctx.enter_context(tc.tile_pool(name="ops", bufs=2, space="PSUM"))

        mk_s = const.tile([BLK, 2 * QTW], f16)
        kt_s = const.tile([2 * D, S2], f16)
        vp_s = const.tile([BLK, PAR * 65], f16)
        v8_s = const.tile([BLK, 14, 80], u8)
        qt_s = const.tile([2 * D, S], f16)

        # exp bias const, on the otherwise-free vector engine so nothing
        # upstream delays the first ACT/DVE exp
        bias_t = const.tile([BLK, 1], f32, name="biasln2")
        nc.vector.memset(bias_t[:], -PSHIFT * LN2)
        scr_t = const.tile([BLK, 1], f32, name="scr")

        # Input DMAs in first-use order (tile 7 body pairs need kt blocks
        # 0.. and qt tile 7 first). A dummy 1-element exp right after the
        # first DMA issue pulls the ~1.3us ACT table load off the critical
        # path (it would otherwise precede the first real exp).
        T0 = tile_order[0]
        nc.scalar.dma_start(kt_s[:, 0:256], kt_d[:, 0:256])
        nc.scalar.activation(scr_t[:], bias_t[:], Exp, scale=0.125)
        nc.sync.dma_start(qt_s[:, T0 * QTW : (T0 + 1) * QTW],
                          qt_d[:, T0 * QTW : (T0 + 1) * QTW])
        nc.gpsimd.dma_start(v8_s[:], v8_d[:])
        nc.scalar.dma_start(kt_s[:, 256:], kt_d[:, 256:])
        nc.sync.dma_start(qt_s[:, 0 : T0 * QTW], qt_d[:, 0 : T0 * QTW])
        nc.gpsimd.dma_start(vp_s[:], vp_d[:])
        nc.scalar.dma_start(mk_s[:], mk_d[:])

        # PE warmup: ~5us of continuous PE busy (warmups + first real
        # pairs, gap-free) steps the clock up ~2x (measured); the warmups
        # overlap the input-DMA window.
        wsrc = const.tile([BLK, QTW], f16, name="wsrc")
        nc.vector.memset(wsrc[:], 0.0)
        wps = sc_ps.tile([BLK, 2 * QTW], f32, tag="sc", name="wps")
        for _ in range(WARMUP_MMS):
            nc.tensor.matmul(wps[:, 0:QTW], wsrc[:, 0:BLK], wsrc[:], start=True, stop=True)

        # ---- schedule construction -------------------------------------
        jobs = []
        dve_err = 0
        for ti, T in enumerate(tile_order):
            depth = 2 * T + 2
            body = list(range(0, depth - 2, 2))
            pair_lo = body + [depth - 2] if ti == 0 else [depth - 2] + body
            for pi, lo in enumerate(pair_lo):
                if lo == depth - 2:
                    kind = "bnd"
                elif T == 1:
                    kind = "fp16"
                elif ti == len(tile_order) - 1 and pi == len(pair_lo) - 1:
                    kind = "act8"  # cheap tail
                elif len(jobs) < FIRST_DVE:
                    # alternate engines during pipeline fill so the first
                    # exps run in parallel, not serialized on one engine
                    kind = "dve8" if len(jobs) % 2 == 0 else "act8"
                else:
                    dve_err += DVE_NUM
                    if dve_err >= DVE_DEN:
                        dve_err -= DVE_DEN
                        kind = "dve8"
                    else:
                        kind = "act8"
                jobs.append(dict(T=T, ti=ti, lo=lo, kind=kind,
                                 last=pi == len(pair_lo) - 1))
        n_total = {}
        for j in jobs:
            n_total[j["T"]] = n_total.get(j["T"], 0) + (2 if j["kind"] in ("bnd", "fp16") else 1)

        ops_t, n_mm2 = {}, {}
        copy_ctr = 0

        def emit_scores(j):
            """fp16 row-tiled score pair (2 kv blocks concurrently)."""
            T, lo = j["T"], j["lo"]
            boundary = j["kind"] == "bnd"
            sc = sc_ps.tile([BLK, 2 * QTW], f32, tag="sc")
            wid = (QTW, QTW // 2) if boundary else (QTW, QTW)
            for k, rg in ((0, 0), (1, D)):
                blk = lo + k
                nc.tensor.matmul(
                    sc[:, k * QTW : k * QTW + wid[k]],
                    kt_s[rg : rg + D, blk * BLK : (blk + 1) * BLK],
                    qt_s[rg : rg + D, T * QTW + (QTW - wid[k]) : (T + 1) * QTW],
                    start=True, stop=True,
                    tile_position=(rg, 0),
                )
            j["sc"], j["wid"] = sc, wid

        final_T = tile_order[-1]
        deferred = []  # (target_ji, pv_thunk): boundary PVs emitted late

        def pv(T, ti, out_cols, lhsT, rhs, dr=False):
            nonlocal copy_ctr
            if T not in ops_t:
                ops_t[T] = out_ps.tile([65, QTW], f32, tag="ops", name=f"ops{T}")
                n_mm2[T] = 0
            ops = ops_t[T]
            n_mm2[T] += 1
            done = n_mm2[T] == n_total[T]
            nc.tensor.matmul(
                ops[:, out_cols[0] : out_cols[1]], lhsT, rhs,
                start=(n_mm2[T] == 1), stop=done,
                perf_mode=DR if dr else None,
            )
            if done:
                osb = opool.tile([65, QTW], f32, tag="osb", name=f"osb{T}")
                if T == final_T or copy_ctr % 2 == 1:
                    nc.scalar.copy(osb[:], ops[:])
                else:
                    nc.vector.tensor_copy(osb[:], ops[:])
                copy_ctr += 1
                nc.sync.dma_start(out_d[:, T * QTW : (T + 1) * QTW], osb[:])

        def emit_rest(j, ji):
            T, ti, lo, sc, wid = j["T"], j["ti"], j["lo"], j["sc"], j["wid"]
            if j["kind"] == "bnd":
                pt = p16pool.tile([BLK, 2 * QTW], f16, tag="pt")
                ew = QTW + wid[1]
                nc.scalar.activation(pt[:, 0:ew], sc[:, 0:ew], Exp, scale=0.125, bias=bias_t[:])
                nc.gpsimd.tensor_mul(pt[:, 0:256], pt[:, 0:256], mk_s[:, 0:256])
                nc.gpsimd.tensor_mul(
                    pt[:, QTW:ew], pt[:, QTW:ew], mk_s[:, QTW + 256 : 2 * QTW]
                )

                def bnd_pv(pt=pt, T=T, ti=ti, lo=lo, wid=wid):
                    for k in (0, 1):
                        pv(T, ti, (QTW - wid[k], QTW),
                           vp_s[:, (lo + k) * 65 : (lo + k + 1) * 65],
                           pt[:, k * QTW : k * QTW + wid[k]])
                # defer the boundary PV so the exp -> gpsimd-mask chain
                # (~2.3us) never stalls the in-order PE queue; the final
                # tile's boundary goes last of all (cheapest tail chain:
                # no exp between the last PV and the output copy)
                tgt = len(jobs) + 1 if T == final_T else ji + 2
                deferred.append((tgt, bnd_pv))
            elif j["kind"] == "fp16":
                ptd = p16pool.tile([BLK, 2 * QTW], mybir.dt.uint16, tag="ptd")
                nc.vector.tensor_scalar(
                    ptd[:, :], sc[:, :], TRICK_C0, TRICK_C1, MUL, ADD,
                )
                for k in (0, 1):
                    pv(T, ti, (0, QTW),
                       vp_s[:, (lo + k) * 65 : (lo + k + 1) * 65],
                       ptd.bitcast(f16)[:, k * QTW : (k + 1) * QTW])
            elif j["kind"] == "dve8":
                p8 = p8pool.tile([BLK, 2, QTW], u8, tag="p8")
                nc.vector.tensor_scalar(
                    p8[:, :, :], sc[:, :].rearrange("p (two f) -> p two f", two=2),
                    LOG2E, TRICK8_C1, MUL, ADD,
                )
                pv(T, ti, (0, QTW), v8_s[:, lo : lo + 2, 0:65].bitcast(f8),
                   p8[:, :, :].bitcast(f8), dr=True)
            else:  # act8
                p8 = p8pool.tile([BLK, 2, QTW], u8, tag="p8")
                nc.scalar.activation(
                    p8[:, :, :].bitcast(f8),
                    sc[:, :].rearrange("p (two f) -> p two f", two=2),
                    Exp, scale=0.125, bias=bias_t[:],
                )
                pv(T, ti, (0, QTW), v8_s[:, lo : lo + 2, 0:65].bitcast(f8),
                   p8[:, :, :].bitcast(f8), dr=True)

        # pipelined emission, lookahead 2: scores of pairs j+1, j+2 precede
        # the PV of pair j in the in-order PE stream, so the PE never
        # head-of-line blocks waiting for pair j's exp (sc pool has exactly
        # 3 bufs = the 3 live score tiles).
        emit_scores(jobs[0])
        emit_scores(jobs[1])
        for ji, j in enumerate(jobs):
            if ji + 2 < len(jobs):
                emit_scores(jobs[ji + 2])
            # fire due deferred boundary PVs BEFORE this job's exp (its pt
            # buffer reuse must not precede the deferred reader)
            for tgt, thunk in [dd for dd in deferred if dd[0] <= ji]:
                deferred.remove((tgt, thunk))
                thunk()
            emit_rest(j, ji)
        for _, thunk in deferred:
            thunk()

    nc.compile()
    _prog_cache["nc"] = nc
    return nc


def _make_masks(h):
    """[128, 1024] fp16 multiplicative (1=keep, 0=masked) masks: two stacked
    tiles for the 2nd-to-last / last parity-kv loop positions of every q tile
    (relative diagonal offsets r = h and r = h + 2)."""
    tri = (np.arange(QTW)[None, :BLK] >= np.arange(BLK)[:, None]).astype(np.float16)
    full = np.zeros((BLK, BLK), dtype=np.float16)
    keep = np.ones((BLK, BLK), dtype=np.float16)

    def mask_for_r(r):
        cols = []
        for cb in range(QTW // BLK):
            if cb < r:
                cols.append(full)
            elif cb == r:
                cols.append(tri)
            else:
                cols.append(keep)
        return np.concatenate(cols, axis=1)  # [128, 512]

    return np.concatenate([mask_for_r(h), mask_for_r(h + 2)], axis=1)


def kernel(query, key, value, padding):
    import ml_dtypes

    f8 = ml_dtypes.float8_e4m3fn
    query = np.asarray(query, dtype=np.float32)
    key = np.asarray(key, dtype=np.float32)
    value = np.asarray(value, dtype=np.float32)
    padding = np.asarray(padding, dtype=bool)

    nc = _build_program()

    in_maps = []
    for c in range(NCORES):
        b, h = divmod(c, 2)
        qt1 = np.ascontiguousarray(query[b].T).astype(np.float16)  # [64, 4096]
        qt = np.concatenate([qt1, qt1], axis=0)  # [128, 4096] row-tiling dup
        blocks = [2 * i + h for i in range(PAR)]
        perm = np.concatenate([np.arange(BLK * j, BLK * (j + 1)) for j in blocks])
        kperm = key[b][perm]  # [2048, 64]
        kt1 = np.ascontiguousarray(kperm.T).astype(np.float16)  # [64, 2048]
        kt = np.concatenate([kt1, kt1], axis=0)  # [128, 2048]
        vp = np.zeros((BLK, PAR * 65), dtype=np.float16)
        vblks = []
        for i, j in enumerate(blocks):
            vblk = value[b, BLK * j : BLK * (j + 1), :].copy()
            pblk = padding[b, BLK * j : BLK * (j + 1)]
            vblk[pblk] = 0.0
            ones = np.where(pblk, 0.0, 1.0).astype(np.float32)
            vp[:, 65 * i : 65 * i + 64] = vblk
            vp[:, 65 * i + 64] = ones
            vblks.append((vblk, ones))
        v8 = np.zeros((BLK, 14, 80), dtype=f8)
        for p in range(7):
            for s_ in range(2):
                vblk, ones = vblks[2 * p + s_]
                v8[:, 2 * p + s_, 0:64] = vblk.astype(f8)
                v8[:, 2 * p + s_, 64] = ones.astype(f8)
        in_maps.append({
            "qt": qt, "kt": kt, "vp": vp,
            "v8": v8.view(np.uint8), "mk": _make_masks(h),
        })

    global _last_in_maps
    _last_in_maps = in_maps
    res = run_bass_kernel_spmd(nc, in_maps, list(range(NCORES)))

    out = np.empty((B, S, D), dtype=np.float32)
    for b in range(B):
        r0 = res.results[2 * b]["out"].astype(np.float64)
        r1 = res.results[2 * b + 1]["out"].astype(np.float64)
        num = r0[:64] + r1[:64]
        den = r0[64] + r1[64]
        out[b] = (num / den).T.astype(np.float32)
    return out


# revision 21
# speedup vs baseline: 1.1874x; 1.0025x over previous
"""Causal attention (B=4, S=4096, D=64, fp32) on 8 Trainium2 NeuronCores.

Strategy (v4)
-------------
Sharding: 2 cores per batch element; the two cores of a batch split the KV
blocks by parity (even / odd 128-row blocks). Each core computes, for every
query position of its batch, the *unnormalized* attention numerator and the
softmax denominator contribution of its own KV half; the host sums and
divides (exact: softmax with no max-subtraction).

The baseline was ACT-engine bound (~42us of exp) with the PE at mid clock.
v4 splits exp across ACT+DVE and shrinks the PE stream with fp8 DoubleRow
matmuls on both the scores and PV sides:

  - scores, BODY pairs: fp8 DoubleRow matmuls, 4-way row-tiled (32-partition
    row groups, 2 fp8 values packed per partition = contraction 64): FOUR kv
    blocks (2 pairs) computed concurrently in one PE slot. Boundary / tile-1
    pairs keep fp16 scores (row-tiled pairs) for accuracy.
  - exp, BODY pairs -> P in fp8e4m3 scaled by 2^-PSHIFT, either by
      * ACT: activation(Exp, scale=1/8, bias=-PSHIFT*ln2) -> fp8 out, or
      * DVE: Schraudolph bit-trick in ONE tensor_scalar pass:
        uint8(x*log2e + (7-PSHIFT+delta)*8) whose e4m3 bit pattern IS
        ~exp(x/8)*2^-PSHIFT (uint8 saturation maps very negative scores
        to P=0). Split ratio balances ACT vs DVE busy time.
  - exp, BOUNDARY (diagonal) pairs: accurate ACT exp -> fp16; 0/1 causal
    masks multiplied on the otherwise-idle GPSIMD engine. Early rows (few
    kv terms, no error averaging) only ever see this path. Tile 1's single
    body pair also stays fp16 (DVE fp16-domain trick + fp16 V): it feeds
    the lowest-N body rows, where fp8 V quantization error would dominate.
  - PV: body pairs use ONE fp8 DoubleRow matmul per pair (2 kv blocks
    contracted at once): lhsT = [V_even|V_odd|ones] as [128, 2, 65] fp8
    (stride 80 for the step%16 rule), rhs = P pair [128, 2, 512] fp8.
    Row 64 of the accumulator is sum(P) = softmax denominator. Padded keys:
    V rows and ones entries zeroed host-side (exact).
  - The schedule is a flat list of pair jobs, software-pipelined on the PE:
    scores of the next group are emitted before the PV of the current pair,
    so the in-order PE queue never head-of-line blocks on exp.
  - No PE warmup: with DMA+all engines active this device pins the PE at
    mid clock regardless (measured), so warmup matmuls only delay the
    first real pair.
Host: packs per-core fp16/fp8 inputs, combines/normalizes outputs.
"""

import numpy as np
from contextlib import ExitStack

import concourse.tile as tile
from concourse import bacc, mybir
from concourse.bass_utils import run_bass_kernel_spmd

B, S, D = 4, 4096, 64
NCORES = 8
BLK = 128            # kv block rows
QTW = 512            # q tile width
NQT = S // QTW       # 8 q tiles
PAR = S // BLK // 2  # 16 kv blocks per parity half
S2 = PAR * BLK       # 2048 kv columns per core

LN2 = 0.6931471805599453
PSHIFT = 3          # P scaled by 2^-PSHIFT: fp8 saturation needs s>61.3
                    # (max score 59.4) with small fp8-subnormal mass
LOG2E = 1.4426950408889634
TRICK_C1 = 12242.93  # fp16 trick: (15 - PSHIFT + delta)*1024, delta=-0.0440
TRICK_C0 = 184.6649652  # 0.125 * log2(e) * 1024
TRICK8_C1 = 31.651   # fp8 trick: (7 - PSHIFT + delta)*8, delta=-0.0436
DVE_NUM, DVE_DEN = 5, 9  # share of later fp8 body pairs on DVE (Bresenham)
FIRST_DVE = 4        # first body pairs alternate DVE/ACT (parallel fill)
WARMUP_MMS = 5       # PE clock-ramp warmup matmuls (bridge to data-ready)

_prog_cache = {}


def _build_program():
    if "nc" in _prog_cache:
        return _prog_cache["nc"]
    nc = bacc.Bacc("TRN2", target_bir_lowering=False, debug=False, num_devices=NCORES)
    f32, f16 = mybir.dt.float32, mybir.dt.float16
    u8, f8 = mybir.dt.uint8, mybir.dt.float8e4
    Exp = mybir.ActivationFunctionType.Exp
    MUL, ADD = mybir.AluOpType.mult, mybir.AluOpType.add
    DR = mybir.MatmulPerfMode.DoubleRow

    qt_d = nc.dram_tensor("qt", [2 * D, S], f16, kind="ExternalInput").ap()
    kt_d = nc.dram_tensor("kt", [2 * D, S2], f16, kind="ExternalInput").ap()
    vp_d = nc.dram_tensor("vp", [BLK, PAR * 65], f16, kind="ExternalInput").ap()
    # body-pair fp8 PV weights: 7 pairs x [V_even | V_odd | ones] packed
    # [128, 14, 80] (pair p at [:, 2p:2p+2, :], cols 0:64 V, col 64 ones)
    v8_d = nc.dram_tensor("v8", [BLK, 14, 80], u8, kind="ExternalInput").ap()
    mk_d = nc.dram_tensor("mk", [BLK, 2 * QTW], f16, kind="ExternalInput").ap()
    out_d = nc.dram_tensor("out", [65, S], f32, kind="ExternalOutput").ap()

    # Process deepest tile first (absorbs pipeline fill), T2 last (its tail
    # pair is a cheap ACT fp8 one). Tile 1 mid-schedule.
    tile_order = [7, 0, 6, 5, 4, 3, 1, 2]

    with tile.TileContext(nc) as tc, ExitStack() as ctx:
        const = ctx.enter_context(tc.tile_pool(name="const", bufs=1))
        p16pool = ctx.enter_context(tc.tile_pool(name="p16", bufs=2))
        p8pool = ctx.enter_context(tc.tile_pool(name="p8", bufs=3))
        opool = ctx.enter_context(tc.tile_pool(name="op", bufs=3))
        sc_ps = ctx.enter_context(tc.tile_pool(name="scps", bufs=3, space="PSUM"))
        out_ps = ctx.enter_context(tc.tile_pool(name="ops", bufs=2, space="PSUM"))

        mk_s = const.tile([BLK, 2 * QTW], f16)
        kt_s = const.tile([2 * D, S2], f16)
        vp_s = const.tile([BLK, PAR * 65], f16)
        v8_s = const.tile([BLK, 14, 80], u8)
        qt_s = const.tile([2 * D, S], f16)

        # exp bias const, on the otherwise-free vector engine so nothing
        # upstream delays the first ACT/DVE exp
        bias_t = const.tile([BLK, 1], f32, name="biasln2")
        nc.vector.memset(bias_t[:], -PSHIFT * LN2)
        scr_t = const.tile([BLK, 1], f32, name="scr")

        # Input DMAs in first-use order (tile 7 body pairs need kt blocks
        # 0.. and qt tile 7 first). A dummy 1-element exp right after the
        # first DMA issue pulls the ~1.3us ACT table load off the critical
        # path (it would otherwise precede the first real exp).
        T0 = tile_order[0]
        nc.scalar.dma_start(kt_s[:, 0:256], kt_d[:, 0:256])
        nc.scalar.activation(scr_t[:], bias_t[:], Exp, scale=0.125)
        nc.sync.dma_start(qt_s[:, T0 * QTW : (T0 + 1) * QTW],
                          qt_d[:, T0 * QTW : (T0 + 1) * QTW])
        # v8 gates every body PV -- keep it on the fast scalar HWDGE ring
        nc.scalar.dma_start(v8_s[:], v8_d[:])
        nc.scalar.dma_start(kt_s[:, 256:], kt_d[:, 256:])
        nc.sync.dma_start(qt_s[:, 0 : T0 * QTW], qt_d[:, 0 : T0 * QTW])
        nc.gpsimd.dma_start(vp_s[:], vp_d[:])
        nc.gpsimd.dma_start(mk_s[:], mk_d[:])

        # PE warmup: ~5us of continuous PE busy (warmups + first real
        # pairs, gap-free) steps the clock up ~2x (measured); the warmups
        # overlap the input-DMA window.
        wsrc = const.tile([BLK, QTW], f16, name="wsrc")
        nc.vector.memset(wsrc[:], 0.0)
        wps = sc_ps.tile([BLK, 2 * QTW], f32, tag="sc", name="wps")
        for _ in range(WARMUP_MMS):
            nc.tensor.matmul(wps[:, 0:QTW], wsrc[:, 0:BLK], wsrc[:], start=True, stop=True)

        # ---- schedule construction -------------------------------------
        jobs = []
        dve_err = 0
        for ti, T in enumerate(tile_order):
            depth = 2 * T + 2
            body = list(range(0, depth - 2, 2))
            pair_lo = body + [depth - 2] if ti == 0 else [depth - 2] + body
            for pi, lo in enumerate(pair_lo):
                if lo == depth - 2:
                    kind = "bnd"
                elif T == 1:
                    kind = "fp16"
                elif ti == len(tile_order) - 1 and pi == len(pair_lo) - 1:
                    kind = "act8"  # cheap tail
                elif len(jobs) < FIRST_DVE:
                    # alternate engines during pipeline fill so the first
                    # exps run in parallel, not serialized on one engine
                    kind = "dve8" if len(jobs) % 2 == 0 else "act8"
                else:
                    dve_err += DVE_NUM
                    if dve_err >= DVE_DEN:
                        dve_err -= DVE_DEN
                        kind = "dve8"
                    else:
                        kind = "act8"
                jobs.append(dict(T=T, ti=ti, lo=lo, kind=kind,
                                 last=pi == len(pair_lo) - 1))
        n_total = {}
        for j in jobs:
            n_total[j["T"]] = n_total.get(j["T"], 0) + (2 if j["kind"] in ("bnd", "fp16") else 1)

        ops_t, n_mm2 = {}, {}
        copy_ctr = 0

        def emit_scores(j):
            """fp16 row-tiled score pair (2 kv blocks concurrently)."""
            T, lo = j["T"], j["lo"]
            boundary = j["kind"] == "bnd"
            sc = sc_ps.tile([BLK, 2 * QTW], f32, tag="sc")
            wid = (QTW, QTW // 2) if boundary else (QTW, QTW)
            for k, rg in ((0, 0), (1, D)):
                blk = lo + k
                nc.tensor.matmul(
                    sc[:, k * QTW : k * QTW + wid[k]],
                    kt_s[rg : rg + D, blk * BLK : (blk + 1) * BLK],
                    qt_s[rg : rg + D, T * QTW + (QTW - wid[k]) : (T + 1) * QTW],
                    start=True, stop=True,
                    tile_position=(rg, 0),
                )
            j["sc"], j["wid"] = sc, wid

        final_T = tile_order[-1]
        deferred = []  # (target_ji, pv_thunk): boundary PVs emitted late

        def pv(T, ti, out_cols, lhsT, rhs, dr=False):
            nonlocal copy_ctr
            if T not in ops_t:
                ops_t[T] = out_ps.tile([65, QTW], f32, tag="ops", name=f"ops{T}")
                n_mm2[T] = 0
            ops = ops_t[T]
            n_mm2[T] += 1
            done = n_mm2[T] == n_total[T]
            nc.tensor.matmul(
                ops[:, out_cols[0] : out_cols[1]], lhsT, rhs,
                start=(n_mm2[T] == 1), stop=done,
                perf_mode=DR if dr else None,
            )
            if done:
                osb = opool.tile([65, QTW], f32, tag="osb", name=f"osb{T}")
                if T == final_T or copy_ctr % 2 == 1:
                    nc.scalar.copy(osb[:], ops[:])
                else:
                    nc.vector.tensor_copy(osb[:], ops[:])
                copy_ctr += 1
                nc.sync.dma_start(out_d[:, T * QTW : (T + 1) * QTW], osb[:])

        def emit_rest(j, ji):
            T, ti, lo, sc, wid = j["T"], j["ti"], j["lo"], j["sc"], j["wid"]
            if j["kind"] == "bnd":
                pt = p16pool.tile([BLK, 2 * QTW], f16, tag="pt")
                ew = QTW + wid[1]
                nc.scalar.activation(pt[:, 0:ew], sc[:, 0:ew], Exp, scale=0.125, bias=bias_t[:])
                nc.gpsimd.tensor_mul(pt[:, 0:256], pt[:, 0:256], mk_s[:, 0:256])
                nc.gpsimd.tensor_mul(
                    pt[:, QTW:ew], pt[:, QTW:ew], mk_s[:, QTW + 256 : 2 * QTW]
                )

                def bnd_pv(pt=pt, T=T, ti=ti, lo=lo, wid=wid):
                    for k in (0, 1):
                        pv(T, ti, (QTW - wid[k], QTW),
                           vp_s[:, (lo + k) * 65 : (lo + k + 1) * 65],
                           pt[:, k * QTW : k * QTW + wid[k]])
                # defer the boundary PV so the exp -> gpsimd-mask chain
                # (~2.3us) never stalls the in-order PE queue; the final
                # tile's boundary goes last of all (cheapest tail chain:
                # no exp between the last PV and the output copy)
                tgt = len(jobs) + 1 if T == final_T else ji + 2
                deferred.append((tgt, bnd_pv))
            elif j["kind"] == "fp16":
                ptd = p16pool.tile([BLK, 2 * QTW], mybir.dt.uint16, tag="ptd")
                nc.vector.tensor_scalar(
                    ptd[:, :], sc[:, :], TRICK_C0, TRICK_C1, MUL, ADD,
                )
                for k in (0, 1):
                    pv(T, ti, (0, QTW),
                       vp_s[:, (lo + k) * 65 : (lo + k + 1) * 65],
                       ptd.bitcast(f16)[:, k * QTW : (k + 1) * QTW])
            elif j["kind"] == "dve8":
                p8 = p8pool.tile([BLK, 2, QTW], u8, tag="p8")
                nc.vector.tensor_scalar(
                    p8[:, :, :], sc[:, :].rearrange("p (two f) -> p two f", two=2),
                    LOG2E, TRICK8_C1, MUL, ADD,
                )
                pv(T, ti, (0, QTW), v8_s[:, lo : lo + 2, 0:65].bitcast(f8),
                   p8[:, :, :].bitcast(f8), dr=True)
            else:  # act8
                p8 = p8pool.tile([BLK, 2, QTW], u8, tag="p8")
                nc.scalar.activation(
                    p8[:, :, :].bitcast(f8),
                    sc[:, :].rearrange("p (two f) -> p two f", two=2),
                    Exp, scale=0.125, bias=bias_t[:],
                )
                pv(T, ti, (0, QTW), v8_s[:, lo : lo + 2, 0:65].bitcast(f8),
                   p8[:, :, :].bitcast(f8), dr=True)

        # pipelined emission, lookahead 2: scores of pairs j+1, j+2 precede
        # the PV of pair j in the in-order PE stream, so the PE never
        # head-of-line blocks waiting for pair j's exp (sc pool has exactly
        # 3 bufs = the 3 live score tiles).
        emit_scores(jobs[0])
        emit_scores(jobs[1])
        for ji, j in enumerate(jobs):
            if ji + 2 < len(jobs):
                emit_scores(jobs[ji + 2])
            # fire due deferred boundary PVs BEFORE this job's exp (its pt
            # buffer reuse must not precede the deferred reader)
            for tgt, thunk in [dd for dd in deferred if dd[0] <= ji]:
                deferred.remove((tgt, thunk))
                thunk()
            emit_rest(j, ji)
        for _, thunk in deferred:
            thunk()

    nc.compile()
    _prog_cache["nc"] = nc
    return nc


def _make_masks(h):
    """[128, 1024] fp16 multiplicative (1=keep, 0=masked) masks: two stacked
    tiles for the 2nd-to-last / last parity-kv loop positions of every q tile
    (relative diagonal offsets r = h and r = h + 2)."""
    tri = (np.arange(QTW)[None, :BLK] >= np.arange(BLK)[:, None]).astype(np.float16)
    full = np.zeros((BLK, BLK), dtype=np.float16)
    keep = np.ones((BLK, BLK), dtype=np.float16)

    def mask_for_r(r):
        cols = []
        for cb in range(QTW // BLK):
            if cb < r:
                cols.append(full)
            elif cb == r:
                cols.append(tri)
            else:
                cols.append(keep)
        return np.concatenate(cols, axis=1)  # [128, 512]

    return np.concatenate([mask_for_r(h), mask_for_r(h + 2)], axis=1)


def kernel(query, key, value, padding):
    import ml_dtypes

    f8 = ml_dtypes.float8_e4m3fn
    query = np.asarray(query, dtype=np.float32)
    key = np.asarray(key, dtype=np.float32)
    value = np.asarray(value, dtype=np.float32)
    padding = np.asarray(padding, dtype=bool)

    nc = _build_program()

    in_maps = []
    for c in range(NCORES):
        b, h = divmod(c, 2)
        qt1 = np.ascontiguousarray(query[b].T).astype(np.float16)  # [64, 4096]
        qt = np.concatenate([qt1, qt1], axis=0)  # [128, 4096] row-tiling dup
        blocks = [2 * i + h for i in range(PAR)]
        perm = np.concatenate([np.arange(BLK * j, BLK * (j + 1)) for j in blocks])
        kperm = key[b][perm]  # [2048, 64]
        kt1 = np.ascontiguousarray(kperm.T).astype(np.float16)  # [64, 2048]
        kt = np.concatenate([kt1, kt1], axis=0)  # [128, 2048]
        vp = np.zeros((BLK, PAR * 65), dtype=np.float16)
        vblks = []
        for i, j in enumerate(blocks):
            vblk = value[b, BLK * j : BLK * (j + 1), :].copy()
            pblk = padding[b, BLK * j : BLK * (j + 1)]
            vblk[pblk] = 0.0
            ones = np.where(pblk, 0.0, 1.0).astype(np.float32)
            vp[:, 65 * i : 65 * i + 64] = vblk
            vp[:, 65 * i + 64] = ones
            vblks.append((vblk, ones))
        v8 = np.zeros((BLK, 14, 80), dtype=f8)
        for p in range(7):
            for s_ in range(2):
                vblk, ones = vblks[2 * p + s_]
                v8[:, 2 * p + s_, 0:64] = vblk.astype(f8)
                v8[:, 2 * p + s_, 64] = ones.astype(f8)
        in_maps.append({
            "qt": qt, "kt": kt, "vp": vp,
            "v8": v8.view(np.uint8), "mk": _make_masks(h),
        })

    global _last_in_maps
    _last_in_maps = in_maps
    res = run_bass_kernel_spmd(nc, in_maps, list(range(NCORES)))

    out = np.empty((B, S, D), dtype=np.float32)
    for b in range(B):
        r0 = res.results[2 * b]["out"].astype(np.float64)
        r1 = res.results[2 * b + 1]["out"].astype(np.float64)
        num = r0[:64] + r1[:64]
        den = r0[64] + r1[64]
        out[b] = (num / den).T.astype(np.float32)
    return out
